# revision 3
# baseline (speedup 1.0000x reference)
"""Multi-head self-attention (AdaptiveTemporalContrastEnhancement) on 8 TRN2 cores.

Key facts baked in:
- The temporal-difference bias delta_c is added uniformly along the softmax
  axis, so softmax cancels it exactly -> it is skipped entirely.
- max |logit| ~ 1.9, so softmax runs without max-subtraction.
- V bias + output bias fold into one effective output bias:
      out = A@ (XWv^T + bv) Wo^T + bo = A@(XWv^T)Wo^T + (Wo bv + bo).
- Data parallel over the 16 (b,t) slices: 2 slices per core, no collectives.
- All matmuls in bf16 (1 cyc/row on PE); accumulation fp32 in PSUM.

Device layout per slice (all "T" = dim-major, tokens along the free axis):
  XT  [d, n]   : 4 x [128, 1024] sbuf tiles (host pre-transposed)
  QT,KT [e, n] : computed as W^T.T @ XT  (4 x [128,1024])
  V   [n, e]   : token-major, computed as XT.T @ WvT, stored with a ones
                 column per head ([128, 8*65] per kv tile) so the PV matmul
                 also produces the softmax denominator (row 64 of Z psum).
  S^T [kv, q]  : per (head, kv-tile) [128, 1024] psum; exp on ACT (scale=1/8)
  Z^T [d, q]   : per head [65, 1024] psum accum over kv tiles; row 64 = denom
  O^T [e, n]   : out-proj from normalized Z^T; host transposes back.
"""

import os
import numpy as np
import ml_dtypes

B, T, N, D = 2, 8, 1024, 512
H, DH = 8, 64
P = 128
NDT = D // P          # 4 d-tiles
NKV = N // P          # 8 kv tiles
NQH = N // 512        # 2 q halves
NCORES = 8
NSLICE = (B * T) // NCORES   # 2 slices per core
S_SCALE = 1.0 / np.sqrt(DH).astype(np.float32)  # 0.125

_CACHE = {}


def _build_nc():
    import concourse.mybir as mybir
    from concourse import bacc
    from concourse.tile import TileContext
    import concourse.bass as bass

    f32, bf16 = mybir.dt.float32, mybir.dt.bfloat16
    nc = bacc.Bacc("TRN2", target_bir_lowering=False, debug=False)

    XT = nc.dram_tensor("XT", [NSLICE, D, N], bf16, kind="ExternalInput")
    WQT = nc.dram_tensor("WQT", [D, D], bf16, kind="ExternalInput")
    WKT = nc.dram_tensor("WKT", [D, D], bf16, kind="ExternalInput")
    WVT = nc.dram_tensor("WVT", [D, D], bf16, kind="ExternalInput")
    WOT = nc.dram_tensor("WOT", [D, D], bf16, kind="ExternalInput")
    BQ = nc.dram_tensor("BQ", [NDT, P, 1], f32, kind="ExternalInput")
    BK = nc.dram_tensor("BK", [NDT, P, 1], f32, kind="ExternalInput")
    BO = nc.dram_tensor("BO", [NDT, P, 1], f32, kind="ExternalInput")
    OT = nc.dram_tensor("OT", [NSLICE, D, N], f32, kind="ExternalOutput")

    Exp = mybir.ActivationFunctionType.Exp

    with TileContext(nc) as tc:
        with (
            tc.tile_pool(name="wpool", bufs=1) as wpool,
            tc.tile_pool(name="xpool", bufs=2) as xpool,
            tc.tile_pool(name="qkpool", bufs=2) as qkpool,
            tc.tile_pool(name="vpool", bufs=2) as vpool,
            tc.tile_pool(name="apool", bufs=3) as apool,
            tc.tile_pool(name="zpool", bufs=2) as zpool,
            tc.tile_pool(name="rpool", bufs=2) as rpool,
            tc.tile_pool(name="opool", bufs=3) as opool,
            tc.tile_pool(name="drpool", bufs=2, space="DRAM") as drpool,
            tc.tile_pool(name="ps_s", bufs=1, space="PSUM") as ps_s,
            tc.tile_pool(name="ps_z", bufs=2, space="PSUM") as ps_z,
            tc.tile_pool(name="ps_po", bufs=2, space="PSUM") as ps_po,
        ):
            # ---- persistent weights / biases ----
            w_sb = {}
            for name, dram in (("wq", WQT), ("wk", WKT), ("wv", WVT), ("wo", WOT)):
                t = wpool.tile([P, NDT * 512], bf16, tag=name)
                for dt_ in range(NDT):
                    nc.sync.dma_start(
                        out=t[:, dt_ * 512:(dt_ + 1) * 512],
                        in_=dram[dt_ * P:(dt_ + 1) * P, :],
                    )
                w_sb[name] = t
            b_sb = {}
            for name, dram in (("bq", BQ), ("bk", BK), ("bo", BO)):
                t = wpool.tile([P, NDT], f32, tag=name)
                for et in range(NDT):
                    nc.sync.dma_start(out=t[:, et:et + 1], in_=dram[et])
                b_sb[name] = t

            for s in range(NSLICE):
                # ---- load X^T ----
                xt = xpool.tile([P, NDT * N], bf16, tag="xt")
                for dt_ in range(NDT):
                    nc.sync.dma_start(
                        out=xt[:, dt_ * N:(dt_ + 1) * N],
                        in_=XT[s, dt_ * P:(dt_ + 1) * P, :],
                    )

                # ---- Q^T / K^T projections ----
                qt = qkpool.tile([P, NDT * N], bf16, tag="qt")
                kt = qkpool.tile([P, NDT * N], bf16, tag="kt")
                for dst, wname, bname in ((qt, "wq", "bq"), (kt, "wk", "bk")):
                    w = w_sb[wname]
                    for et in range(NDT):
                        pss = [ps_po.tile([P, 512], f32, tag="ps_proj", name=f"ps_qk_{s}_{wname}_{et}_{i}") for i in range(NQH)]
                        for dt_ in range(NDT):
                            for qh in range(NQH):
                                nc.tensor.matmul(
                                    pss[qh],
                                    lhsT=w[:, dt_ * 512 + et * P: dt_ * 512 + (et + 1) * P],
                                    rhs=xt[:, dt_ * N + qh * 512: dt_ * N + qh * 512 + 512],
                                    start=(dt_ == 0), stop=(dt_ == NDT - 1),
                                )
                        for qh in range(NQH):
                            nc.vector.tensor_scalar_add(
                                dst[:, et * N + qh * 512: et * N + qh * 512 + 512],
                                pss[qh], b_sb[bname][:, et:et + 1],
                            )

                # ---- V projection (token-major, with ones column per head) ----
                v_sb = vpool.tile([P, NKV * H * 65], bf16, tag="v")
                for kv in range(NKV):
                    blk = v_sb[:, kv * H * 65:(kv + 1) * H * 65]
                    blk3 = blk.rearrange("p (h c) -> p h c", c=65)
                    nc.vector.memset(blk3[:, :, 64:65], 1.0)
                    ps = ps_po.tile([P, 512], f32, tag="ps_proj")
                    for dt_ in range(NDT):
                        nc.tensor.matmul(
                            ps,
                            lhsT=xt[:, dt_ * N + kv * P: dt_ * N + (kv + 1) * P],
                            rhs=w_sb["wv"][:, dt_ * 512:(dt_ + 1) * 512],
                            start=(dt_ == 0), stop=(dt_ == NDT - 1),
                        )
                    nc.vector.tensor_copy(
                        blk3[:, :, 0:64],
                        ps.rearrange("p (h c) -> p h c", c=64),
                    )

                # ---- attention per head ----
                zt = zpool.tile([P, NDT * N], bf16, tag="zt")
                for h in range(H):
                    et, pb = h // 2, 64 * (h % 2)
                    z_ps = ps_z.tile([65, N], f32, tag="z")
                    for kv in range(NKV):
                        s_ps = ps_s.tile([P, N], f32, tag="s")
                        for qh in range(NQH):
                            nc.tensor.matmul(
                                s_ps[:, qh * 512:(qh + 1) * 512],
                                lhsT=kt[pb:pb + 64, et * N + kv * P: et * N + (kv + 1) * P],
                                rhs=qt[pb:pb + 64, et * N + qh * 512: et * N + qh * 512 + 512],
                                start=True, stop=True,
                            )
                        at = apool.tile([P, N], bf16, tag="at")
                        nc.scalar.activation(at, s_ps, Exp, scale=float(S_SCALE))
                        for qh in range(NQH):
                            nc.tensor.matmul(
                                z_ps[:, qh * 512:(qh + 1) * 512],
                                lhsT=v_sb[:, kv * H * 65 + h * 65: kv * H * 65 + (h + 1) * 65],
                                rhs=at[:, qh * 512:(qh + 1) * 512],
                                start=(kv == 0), stop=(kv == NKV - 1),
                            )
                    # softmax denominator -> reciprocal -> broadcast -> normalize
                    rt = rpool.tile([65, N], f32, tag="rrow")
                    nc.vector.reciprocal(rt[64:65, :], z_ps[64:65, :])
                    r_dram = drpool.tile([1, N], f32, tag="rdram")
                    nc.sync.dma_start(out=r_dram, in_=rt[64:65, :])
                    rbc = rpool.tile([64, N], f32, tag="rbc")
                    nc.sync.dma_start(
                        out=rbc,
                        in_=bass.AP(
                            tensor=r_dram.tensor,
                            offset=r_dram.offset,
                            ap=[[0, 64]] + r_dram.ap[1:],
                        ),
                    )
                    nc.vector.tensor_tensor(
                        out=zt[pb:pb + 64, et * N:(et + 1) * N],
                        in0=z_ps[0:64, :], in1=rbc,
                        op=mybir.AluOpType.mult,
                    )

                # ---- output projection O^T = WoT.T @ Z^T + bo_eff ----
                for et in range(NDT):
                    pss = [ps_po.tile([P, 512], f32, tag="ps_proj", name=f"ps_o_{s}_{et}_{i}") for i in range(NQH)]
                    for dd in range(NDT):
                        for qh in range(NQH):
                            nc.tensor.matmul(
                                pss[qh],
                                lhsT=w_sb["wo"][:, dd * 512 + et * P: dd * 512 + (et + 1) * P],
                                rhs=zt[:, dd * N + qh * 512: dd * N + qh * 512 + 512],
                                start=(dd == 0), stop=(dd == NDT - 1),
                            )
                    for qh in range(NQH):
                        o_sb = opool.tile([P, 512], f32, tag="o")
                        nc.vector.tensor_scalar_add(o_sb, pss[qh], b_sb["bo"][:, et:et + 1])
                        nc.sync.dma_start(
                            out=OT[s, et * P:(et + 1) * P, qh * 512:(qh + 1) * 512],
                            in_=o_sb,
                        )

    nc.compile()
    return nc


def _get_nc():
    if "nc" not in _CACHE:
        _CACHE["nc"] = _build_nc()
    return _CACHE["nc"]


def kernel(X, Wq, bq, Wk, bk, Wv, bv, Wo, bo):
    from concourse.bass_utils import run_bass_kernel_spmd

    nc = _get_nc()
    bf16 = ml_dtypes.bfloat16

    Xf = np.asarray(X, np.float32).reshape(B * T, N, D)
    XT_all = np.ascontiguousarray(Xf.transpose(0, 2, 1)).astype(bf16)  # [16, D, N]
    WQT = np.ascontiguousarray(np.asarray(Wq, np.float32).T).astype(bf16)
    WKT = np.ascontiguousarray(np.asarray(Wk, np.float32).T).astype(bf16)
    WVT = np.ascontiguousarray(np.asarray(Wv, np.float32).T).astype(bf16)
    WOT = np.ascontiguousarray(np.asarray(Wo, np.float32).T).astype(bf16)
    bo_eff = (np.asarray(bo, np.float32)
              + np.asarray(Wo, np.float32) @ np.asarray(bv, np.float32))
    BQa = np.asarray(bq, np.float32).reshape(NDT, P, 1)
    BKa = np.asarray(bk, np.float32).reshape(NDT, P, 1)
    BOa = bo_eff.reshape(NDT, P, 1)

    in_maps = []
    for c in range(NCORES):
        in_maps.append({
            "XT": np.ascontiguousarray(XT_all[c * NSLICE:(c + 1) * NSLICE]),
            "WQT": WQT, "WKT": WKT, "WVT": WVT, "WOT": WOT,
            "BQ": BQa, "BK": BKa, "BO": BOa,
        })

    trace = bool(int(os.environ.get("KERNEL_TRACE", "0")))
    kwargs = {}
    if trace:
        import tempfile
        kwargs = {"trace": True, "tmpdir": tempfile.mkdtemp(prefix="ker_trace_")}
    res = run_bass_kernel_spmd(nc, in_maps, core_ids=list(range(NCORES)), **kwargs)
    _CACHE["last_exec_ns"] = res.exec_time_ns

    out = np.empty((B * T, N, D), np.float32)
    for c in range(NCORES):
        ot = np.asarray(res.results[c]["OT"], np.float32)  # [NSLICE, D, N]
        out[c * NSLICE:(c + 1) * NSLICE] = ot.transpose(0, 2, 1)
    return out.reshape(B, T, N, D)


# revision 11
# speedup vs baseline: 1.2970x; 1.2970x over previous
"""Multi-head self-attention (AdaptiveTemporalContrastEnhancement) on 8 TRN2 cores.

Key facts baked in:
- The temporal-difference bias delta_c is added uniformly along the softmax
  axis, so softmax cancels it exactly -> it is skipped entirely.
- max |logit| ~ 1.9, so softmax runs without max-subtraction.
- V bias + output bias fold into one effective output bias:
      out = A@ (XWv^T + bv) Wo^T + bo = A@(XWv^T)Wo^T + (Wo bv + bo).
- Data parallel over the 16 (b,t) slices: 2 slices per core, no collectives.
- All matmuls in bf16 (1 cyc/row on PE); accumulation fp32 in PSUM.

Device layout per slice (all "T" = dim-major, tokens along the free axis):
  XT  [d, n]   : 4 x [128, 1024] sbuf tiles (host pre-transposed)
  QT,KT [e, n] : computed as W^T.T @ XT  (4 x [128,1024])
  V_pad [n, .] : token-major, padded per head to a [128,128] stationary:
                 even head h: V cols 0-63, ones col 64, zeros 65-127
                 odd  head h: ones col 0, zeros 1-63,  V cols 64-127
                 so the PV matmul puts head h's Z^T at partitions 64*(h%2)..+63
                 and the softmax denominator at row 64 (even) / row 0 (odd) --
                 one full-height DVE copy evacuates Z and denominator together.
  S^T [kv, q]  : per (head, kv-tile) [128, 1024] psum; exp on ACT (scale=1/8)
  Z^T [d, q]   : per head [128, 1024] psum accum over kv; copied to sbuf (zun)
  denominators : DMA-gathered to dall[8,1024]; ONE reciprocal per slice;
                 broadcast via DRAM bounce; one TT-mult per head pair.
  O^T [e, n]   : out-proj from normalized Z^T; host transposes back.
"""

import os
import numpy as np
import ml_dtypes

B, T, N, D = 2, 8, 1024, 512
H, DH = 8, 64
P = 128
NDT = D // P          # 4 d-tiles
NKV = N // P          # 8 kv tiles
NQH = N // 512        # 2 q halves
NCORES = 8
NSLICE = (B * T) // NCORES   # 2 slices per core
S_SCALE = float(1.0 / np.sqrt(DH))  # 0.125

_CACHE = {}


def _build_nc():
    import concourse.mybir as mybir
    from concourse import bacc
    from concourse.tile import TileContext
    import concourse.bass as bass

    f32, bf16 = mybir.dt.float32, mybir.dt.bfloat16
    nc = bacc.Bacc("TRN2", target_bir_lowering=False, debug=False)

    XT = nc.dram_tensor("XT", [NSLICE, D, N], bf16, kind="ExternalInput")
    WQT = nc.dram_tensor("WQT", [D, D], bf16, kind="ExternalInput")
    WKT = nc.dram_tensor("WKT", [D, D], bf16, kind="ExternalInput")
    WVT = nc.dram_tensor("WVT", [D, D], bf16, kind="ExternalInput")
    WOT = nc.dram_tensor("WOT", [D, D], bf16, kind="ExternalInput")
    BQ = nc.dram_tensor("BQ", [NDT, P, 1], f32, kind="ExternalInput")
    BK = nc.dram_tensor("BK", [NDT, P, 1], f32, kind="ExternalInput")
    BO = nc.dram_tensor("BO", [NDT, P, 1], f32, kind="ExternalInput")
    OT = nc.dram_tensor("OT", [NSLICE, D, N], f32, kind="ExternalOutput")

    Exp = mybir.ActivationFunctionType.Exp
    Mult = mybir.AluOpType.mult

    with TileContext(nc) as tc:
        with (
            tc.tile_pool(name="wpool", bufs=1) as wpool,
            tc.tile_pool(name="xpool", bufs=2) as xpool,
            tc.tile_pool(name="qkpool", bufs=2) as qkpool,
            tc.tile_pool(name="vpool", bufs=2) as vpool,
            tc.tile_pool(name="apool", bufs=3) as apool,
            tc.tile_pool(name="zpool", bufs=2) as zpool,
            tc.tile_pool(name="rpool", bufs=2) as rpool,
            tc.tile_pool(name="opool", bufs=3) as opool,
            tc.tile_pool(name="drpool", bufs=2, space="DRAM") as drpool,
            tc.tile_pool(name="ps_s", bufs=2, space="PSUM") as ps_s,
            tc.tile_pool(name="ps_z", bufs=2, space="PSUM") as ps_z,
        ):
            # ---- persistent weights / biases ----
            w_sb = {}
            for name, dram in (("wq", WQT), ("wk", WKT), ("wv", WVT), ("wo", WOT)):
                t = wpool.tile([P, NDT * 512], bf16, tag=name, name=f"w_{name}")
                for dt_ in range(NDT):
                    nc.sync.dma_start(
                        out=t[:, dt_ * 512:(dt_ + 1) * 512],
                        in_=dram[dt_ * P:(dt_ + 1) * P, :],
                    )
                w_sb[name] = t
            b_sb = {}
            for name, dram in (("bq", BQ), ("bk", BK), ("bo", BO)):
                t = wpool.tile([P, NDT], f32, tag=name, name=f"b_{name}")
                for et in range(NDT):
                    nc.sync.dma_start(out=t[:, et:et + 1], in_=dram[et])
                b_sb[name] = t

            def load_x(s):
                xt = xpool.tile([P, NDT * N], bf16, tag="xt", name=f"xt_{s}")
                for dt_ in range(NDT):
                    nc.sync.dma_start(
                        out=xt[:, dt_ * N:(dt_ + 1) * N],
                        in_=XT[s, dt_ * P:(dt_ + 1) * P, :],
                    )
                return xt

            def proj_qkv(s, xt):
                qt = qkpool.tile([P, NDT * N], bf16, tag="qt", name=f"qt_{s}")
                kt = qkpool.tile([P, NDT * N], bf16, tag="kt", name=f"kt_{s}")
                for dst, wname, bname in ((qt, "wq", "bq"), (kt, "wk", "bk")):
                    w = w_sb[wname]
                    for et in range(NDT):
                        ps = ps_s.tile([P, N], f32, tag="s", name=f"ps_{wname}_{s}_{et}")
                        for dt_ in range(NDT):
                            for qh in range(NQH):
                                nc.tensor.matmul(
                                    ps[:, qh * 512:(qh + 1) * 512],
                                    lhsT=w[:, dt_ * 512 + et * P: dt_ * 512 + (et + 1) * P],
                                    rhs=xt[:, dt_ * N + qh * 512: dt_ * N + qh * 512 + 512],
                                    start=(dt_ == 0), stop=(dt_ == NDT - 1),
                                )
                        for qh in range(NQH):
                            nc.vector.tensor_scalar_add(
                                dst[:, et * N + qh * 512: et * N + qh * 512 + 512],
                                ps[:, qh * 512:(qh + 1) * 512],
                                b_sb[bname][:, et:et + 1],
                            )

                # V, padded per head to [128,128] stationaries
                v_sb = vpool.tile([P, NKV * H * P], bf16, tag="v", name=f"v_{s}")
                vz = v_sb.rearrange("p (b r) -> p b r", r=256)
                nc.gpsimd.memset(vz[:, :, 65:128], 0.0)    # even-head pad
                nc.gpsimd.memset(vz[:, :, 129:192], 0.0)   # odd-head pad
                nc.vector.memset(vz[:, :, 64:65], 1.0)     # even-head ones col
                nc.vector.memset(vz[:, :, 128:129], 1.0)   # odd-head ones col
                for kv in range(NKV):
                    ps = ps_s.tile([P, N], f32, tag="s", name=f"ps_v_{s}_{kv}")
                    for dt_ in range(NDT):
                        nc.tensor.matmul(
                            ps[:, 0:512],
                            lhsT=xt[:, dt_ * N + kv * P: dt_ * N + (kv + 1) * P],
                            rhs=w_sb["wv"][:, dt_ * 512:(dt_ + 1) * 512],
                            start=(dt_ == 0), stop=(dt_ == NDT - 1),
                        )
                    vblk = v_sb[:, kv * 1024:(kv + 1) * 1024].rearrange(
                        "p (hp r) -> p hp r", r=256)
                    psh = ps[:, 0:512].rearrange("p (hp c) -> p hp c", c=128)
                    nc.vector.tensor_copy(vblk[:, :, 0:64], psh[:, :, 0:64])
                    nc.vector.tensor_copy(vblk[:, :, 192:256], psh[:, :, 64:128])
                return qt, kt, v_sb

            def attention(s, qt, kt, v_sb):
                zun = zpool.tile([P, H * N], bf16, tag="zun", name=f"zun_{s}")
                dall = rpool.tile([H, N], f32, tag="dall", name=f"dall_{s}")
                for h in range(H):
                    et, pb = h // 2, 64 * (h % 2)
                    z_ps = ps_z.tile([P, N], f32, tag="z", name=f"z_{s}_{h}")
                    for kv in range(NKV):
                        s_ps = ps_s.tile([P, N], f32, tag="s", name=f"s_{s}_{h}_{kv}")
                        for qh in range(NQH):
                            nc.tensor.matmul(
                                s_ps[:, qh * 512:(qh + 1) * 512],
                                lhsT=kt[pb:pb + 64, et * N + kv * P: et * N + (kv + 1) * P],
                                rhs=qt[pb:pb + 64, et * N + qh * 512: et * N + qh * 512 + 512],
                                start=True, stop=True,
                            )
                        at = apool.tile([P, N], bf16, tag="at", name=f"at_{s}_{h}_{kv}")
                        nc.scalar.activation(at, s_ps, Exp, scale=S_SCALE)
                        for qh in range(NQH):
                            nc.tensor.matmul(
                                z_ps[:, qh * 512:(qh + 1) * 512],
                                lhsT=v_sb[:, kv * 1024 + h * P: kv * 1024 + (h + 1) * P],
                                rhs=at[:, qh * 512:(qh + 1) * 512],
                                start=(kv == 0), stop=(kv == NKV - 1),
                            )
                    # 65-row copy: Z^T rows + adjacent denominator row -> sbuf;
                    # then a small sbuf->sbuf DMA gathers the denominator.
                    dr = 64 if h % 2 == 0 else 0
                    nc.vector.tensor_copy(
                        zun[:, h * N:(h + 1) * N], z_ps[:, :])
                    nc.gpsimd.dma_start(  # gpsimd: casting DMA bf16 -> f32
                        out=dall[h:h + 1, :], in_=zun[dr:dr + 1, h * N:(h + 1) * N])
                return zun, dall

            def normalize(s, zun, dall):
                zt = zpool.tile([P, NDT * N], bf16, tag="zt", name=f"zt_{s}")
                rall = rpool.tile([H, N], f32, tag="rall", name=f"rall_{s}")
                nc.vector.reciprocal(rall, dall)
                rdram = drpool.tile([H, N], f32, tag="rdram", name=f"rdram_{s}")
                nc.sync.dma_start(out=rdram, in_=rall)
                for j in range(NDT):  # head pairs share one rbc tile
                    rbc = rpool.tile([P, N], f32, tag="rbc", name=f"rbc_{s}_{j}")
                    for p_ in range(2):
                        row = rdram[2 * j + p_: 2 * j + p_ + 1, :]
                        nc.sync.dma_start(
                            out=rbc[64 * p_:64 * p_ + 64, :],
                            in_=bass.AP(tensor=row.tensor, offset=row.offset,
                                        ap=[[0, 64]] + row.ap[1:]),
                        )
                    for p_ in range(2):
                        h = 2 * j + p_
                        pb = 64 * p_
                        nc.vector.tensor_tensor(
                            out=zt[pb:pb + 64, j * N:(j + 1) * N],
                            in0=zun[pb:pb + 64, h * N:(h + 1) * N],
                            in1=rbc[pb:pb + 64, :], op=Mult,
                        )
                return zt

            def out_proj(s, zt):
                for et in range(NDT):
                    ps = ps_s.tile([P, N], f32, tag="s", name=f"ps_o_{s}_{et}")
                    for dd in range(NDT):
                        for qh in range(NQH):
                            nc.tensor.matmul(
                                ps[:, qh * 512:(qh + 1) * 512],
                                lhsT=w_sb["wo"][:, dd * 512 + et * P: dd * 512 + (et + 1) * P],
                                rhs=zt[:, dd * N + qh * 512: dd * N + qh * 512 + 512],
                                start=(dd == 0), stop=(dd == NDT - 1),
                            )
                    for qh in range(NQH):
                        o_sb = opool.tile([P, 512], f32, tag="o", name=f"o_{s}_{et}_{qh}")
                        nc.vector.tensor_scalar_add(
                            o_sb, ps[:, qh * 512:(qh + 1) * 512], b_sb["bo"][:, et:et + 1])
                        nc.sync.dma_start(
                            out=OT[s, et * P:(et + 1) * P, qh * 512:(qh + 1) * 512],
                            in_=o_sb,
                        )

            # software-pipelined phase order across the two slices: the
            # projections of slice 1 fill the PE while slice 0's softmax
            # normalization chain (DVE recip + DMA broadcast) completes.
            xt0 = load_x(0)
            q0, k0, v0 = proj_qkv(0, xt0)
            zun0, dall0 = attention(0, q0, k0, v0)
            xt1 = load_x(1)
            q1, k1, v1 = proj_qkv(1, xt1)
            zt0 = normalize(0, zun0, dall0)
            out_proj(0, zt0)
            zun1, dall1 = attention(1, q1, k1, v1)
            zt1 = normalize(1, zun1, dall1)
            out_proj(1, zt1)

    nc.compile()
    return nc


def _get_nc():
    if "nc" not in _CACHE:
        _CACHE["nc"] = _build_nc()
    return _CACHE["nc"]


def kernel(X, Wq, bq, Wk, bk, Wv, bv, Wo, bo):
    from concourse.bass_utils import run_bass_kernel_spmd

    nc = _get_nc()
    bf16 = ml_dtypes.bfloat16

    Xf = np.asarray(X, np.float32).reshape(B * T, N, D)
    XT_all = np.ascontiguousarray(Xf.transpose(0, 2, 1)).astype(bf16)  # [16, D, N]
    WQT = np.ascontiguousarray(np.asarray(Wq, np.float32).T).astype(bf16)
    WKT = np.ascontiguousarray(np.asarray(Wk, np.float32).T).astype(bf16)
    WVT = np.ascontiguousarray(np.asarray(Wv, np.float32).T).astype(bf16)
    WOT = np.ascontiguousarray(np.asarray(Wo, np.float32).T).astype(bf16)
    bo_eff = (np.asarray(bo, np.float32)
              + np.asarray(Wo, np.float32) @ np.asarray(bv, np.float32))
    BQa = np.asarray(bq, np.float32).reshape(NDT, P, 1)
    BKa = np.asarray(bk, np.float32).reshape(NDT, P, 1)
    BOa = bo_eff.reshape(NDT, P, 1)

    in_maps = []
    for c in range(NCORES):
        in_maps.append({
            "XT": np.ascontiguousarray(XT_all[c * NSLICE:(c + 1) * NSLICE]),
            "WQT": WQT, "WKT": WKT, "WVT": WVT, "WOT": WOT,
            "BQ": BQa, "BK": BKa, "BO": BOa,
        })

    trace = bool(int(os.environ.get("KERNEL_TRACE", "0")))
    kwargs = {}
    if trace:
        import tempfile
        kwargs = {"trace": True, "tmpdir": tempfile.mkdtemp(prefix="ker_trace_")}
    res = run_bass_kernel_spmd(nc, in_maps, core_ids=list(range(NCORES)), **kwargs)
    _CACHE["last_exec_ns"] = res.exec_time_ns

    out = np.empty((B * T, N, D), np.float32)
    for c in range(NCORES):
        ot = np.asarray(res.results[c]["OT"], np.float32)  # [NSLICE, D, N]
        out[c * NSLICE:(c + 1) * NSLICE] = ot.transpose(0, 2, 1)
    return out.reshape(B, T, N, D)


# revision 13
# speedup vs baseline: 1.7662x; 1.3618x over previous
"""Multi-head self-attention (AdaptiveTemporalContrastEnhancement) on 8 TRN2 cores.

Key facts baked in:
- The temporal-difference bias delta_c is added uniformly along the softmax
  axis, so softmax cancels it exactly -> it is skipped entirely.
- max |logit| ~ 1.9, so softmax runs without max-subtraction.
- V bias + output bias fold into one effective output bias:
      out = A@ (XWv^T + bv) Wo^T + bo = A@(XWv^T)Wo^T + (Wo bv + bo).
- Data parallel over the 16 (b,t) slices: 2 slices per core, no collectives.
- All matmuls in bf16 (1 cyc/row on PE); accumulation fp32 in PSUM.

Device layout per slice (all "T" = dim-major, tokens along the free axis):
  XT  [d, n]   : 4 x [128, 1024] sbuf tiles (host pre-transposed)
  QT,KT [e, n] : computed as W^T.T @ XT  (4 x [128,1024])
  V_pad [n, .] : token-major, padded per head to a [128,128] stationary:
                 even head h: V cols 0-63, ones col 64, zeros 65-127
                 odd  head h: ones col 0, zeros 1-63,  V cols 64-127
                 so the PV matmul puts head h's Z^T at partitions 64*(h%2)..+63
                 and the softmax denominator at row 64 (even) / row 0 (odd) --
                 one full-height DVE copy evacuates Z and denominator together.
  S^T [kv, q]  : head-PAIR packed: one [128, 1024] psum tile holds both
                 heads' S^T for one (kv, q-half); the two S matmuls use
                 disjoint PE row groups (partitions 0-63 / 64-127) and run
                 concurrently. 1/sqrt(dh) is folded into WQT host-side.
  Z^T [d, q]   : per head [128, 1024] psum accum over kv; copied to sbuf (zun)
  denominators : DMA-gathered to dall[8,1024]; ONE reciprocal per slice;
                 broadcast via DRAM bounce; one TT-mult per head pair.
  O^T [e, n]   : out-proj from normalized Z^T; host transposes back.
"""

import os
import numpy as np
import ml_dtypes

B, T, N, D = 2, 8, 1024, 512
H, DH = 8, 64
P = 128
NDT = D // P          # 4 d-tiles
NKV = N // P          # 8 kv tiles
NQH = N // 512        # 2 q halves
NCORES = 8
NSLICE = (B * T) // NCORES   # 2 slices per core
S_SCALE = float(1.0 / np.sqrt(DH))  # 0.125

_CACHE = {}


def _build_nc():
    import concourse.mybir as mybir
    from concourse import bacc
    from concourse.tile import TileContext
    import concourse.bass as bass

    f32, bf16 = mybir.dt.float32, mybir.dt.bfloat16
    nc = bacc.Bacc("TRN2", target_bir_lowering=False, debug=False)

    XT = nc.dram_tensor("XT", [NSLICE, D, N], bf16, kind="ExternalInput")
    WQT = nc.dram_tensor("WQT", [D, D], bf16, kind="ExternalInput")
    WKT = nc.dram_tensor("WKT", [D, D], bf16, kind="ExternalInput")
    WVT = nc.dram_tensor("WVT", [D, D], bf16, kind="ExternalInput")
    WOT = nc.dram_tensor("WOT", [D, D], bf16, kind="ExternalInput")
    BQ = nc.dram_tensor("BQ", [NDT, P, 1], f32, kind="ExternalInput")
    BK = nc.dram_tensor("BK", [NDT, P, 1], f32, kind="ExternalInput")
    BO = nc.dram_tensor("BO", [NDT, P, 1], f32, kind="ExternalInput")
    OT = nc.dram_tensor("OT", [NSLICE, D, N], f32, kind="ExternalOutput")

    Exp = mybir.ActivationFunctionType.Exp
    Mult = mybir.AluOpType.mult

    with TileContext(nc) as tc:
        with (
            tc.tile_pool(name="wpool", bufs=1) as wpool,
            tc.tile_pool(name="xpool", bufs=2) as xpool,
            tc.tile_pool(name="qkpool", bufs=2) as qkpool,
            tc.tile_pool(name="vpool", bufs=2) as vpool,
            tc.tile_pool(name="apool", bufs=3) as apool,
            tc.tile_pool(name="zpool", bufs=2) as zpool,
            tc.tile_pool(name="rpool", bufs=2) as rpool,
            tc.tile_pool(name="opool", bufs=3) as opool,
            tc.tile_pool(name="drpool", bufs=2, space="DRAM") as drpool,
            tc.tile_pool(name="ps_s", bufs=2, space="PSUM") as ps_s,
            tc.tile_pool(name="ps_z", bufs=2, space="PSUM") as ps_z,
        ):
            # ---- persistent weights / biases ----
            w_sb = {}
            for name, dram in (("wq", WQT), ("wk", WKT), ("wv", WVT), ("wo", WOT)):
                t = wpool.tile([P, NDT * 512], bf16, tag=name, name=f"w_{name}")
                for dt_ in range(NDT):
                    nc.sync.dma_start(
                        out=t[:, dt_ * 512:(dt_ + 1) * 512],
                        in_=dram[dt_ * P:(dt_ + 1) * P, :],
                    )
                w_sb[name] = t
            b_sb = {}
            for name, dram in (("bq", BQ), ("bk", BK), ("bo", BO)):
                t = wpool.tile([P, NDT], f32, tag=name, name=f"b_{name}")
                for et in range(NDT):
                    nc.sync.dma_start(out=t[:, et:et + 1], in_=dram[et])
                b_sb[name] = t

            def load_x(s):
                xt = xpool.tile([P, NDT * N], bf16, tag="xt", name=f"xt_{s}")
                for dt_ in range(NDT):
                    nc.sync.dma_start(
                        out=xt[:, dt_ * N:(dt_ + 1) * N],
                        in_=XT[s, dt_ * P:(dt_ + 1) * P, :],
                    )
                return xt

            def proj_qk_chunk(s, et, xt, qt, kt):
                for dst, wname, bname in ((qt, "wq", "bq"), (kt, "wk", "bk")):
                    w = w_sb[wname]
                    ps = ps_s.tile([P, N], f32, tag="s", name=f"ps_{wname}_{s}_{et}")
                    for dt_ in range(NDT):
                        for qh in range(NQH):
                            nc.tensor.matmul(
                                ps[:, qh * 512:(qh + 1) * 512],
                                lhsT=w[:, dt_ * 512 + et * P: dt_ * 512 + (et + 1) * P],
                                rhs=xt[:, dt_ * N + qh * 512: dt_ * N + qh * 512 + 512],
                                start=(dt_ == 0), stop=(dt_ == NDT - 1),
                            )
                    for qh in range(NQH):
                        nc.vector.tensor_scalar_add(
                            dst[:, et * N + qh * 512: et * N + qh * 512 + 512],
                            ps[:, qh * 512:(qh + 1) * 512],
                            b_sb[bname][:, et:et + 1],
                        )

            def proj_v(s, xt):
                # V, padded per head to [128,128] stationaries
                v_sb = vpool.tile([P, NKV * H * P], bf16, tag="v", name=f"v_{s}")
                vz = v_sb.rearrange("p (b r) -> p b r", r=256)
                nc.gpsimd.memset(vz[:, :, 65:128], 0.0)    # even-head pad
                nc.gpsimd.memset(vz[:, :, 129:192], 0.0)   # odd-head pad
                nc.vector.memset(vz[:, :, 64:65], 1.0)     # even-head ones col
                nc.vector.memset(vz[:, :, 128:129], 1.0)   # odd-head ones col
                for kv in range(NKV):
                    ps = ps_s.tile([P, N], f32, tag="s", name=f"ps_v_{s}_{kv}")
                    for dt_ in range(NDT):
                        nc.tensor.matmul(
                            ps[:, 0:512],
                            lhsT=xt[:, dt_ * N + kv * P: dt_ * N + (kv + 1) * P],
                            rhs=w_sb["wv"][:, dt_ * 512:(dt_ + 1) * 512],
                            start=(dt_ == 0), stop=(dt_ == NDT - 1),
                        )
                    vblk = v_sb[:, kv * 1024:(kv + 1) * 1024].rearrange(
                        "p (hp r) -> p hp r", r=256)
                    psh = ps[:, 0:512].rearrange("p (hp c) -> p hp c", c=128)
                    nc.vector.tensor_copy(vblk[:, :, 0:64], psh[:, :, 0:64])
                    nc.vector.tensor_copy(vblk[:, :, 192:256], psh[:, :, 64:128])
                return v_sb

            def attention_pair(s, j, qt, kt, v_sb, zun, dall):
                """Heads 2j, 2j+1: S matmuls packed into disjoint PE row
                groups; one exp covers both heads; PV per head."""
                et = j                 # e-tile holding this pair of heads
                zs = [ps_z.tile([P, N], f32, tag="z", name=f"z_{s}_{j}_{p_}")
                      for p_ in range(2)]
                for qh in range(NQH):
                    for kv in range(NKV):
                        s_ps = ps_s.tile([P, N], f32, tag="s", name=f"s_{s}_{j}_{qh}_{kv}")
                        for p_ in range(2):
                            pb = 64 * p_
                            nc.tensor.matmul(
                                s_ps[:, p_ * 512:(p_ + 1) * 512],
                                lhsT=kt[pb:pb + 64, et * N + kv * P: et * N + (kv + 1) * P],
                                rhs=qt[pb:pb + 64, et * N + qh * 512: et * N + qh * 512 + 512],
                                start=True, stop=True,
                            )
                        at = apool.tile([P, N], bf16, tag="at", name=f"at_{s}_{j}_{qh}_{kv}")
                        nc.scalar.activation(at, s_ps, Exp)
                        for p_ in range(2):
                            h = 2 * j + p_
                            nc.tensor.matmul(
                                zs[p_][:, qh * 512:(qh + 1) * 512],
                                lhsT=v_sb[:, kv * 1024 + h * P: kv * 1024 + (h + 1) * P],
                                rhs=at[:, p_ * 512:(p_ + 1) * 512],
                                start=(kv == 0), stop=(kv == NKV - 1),
                            )
                for p_ in range(2):
                    h = 2 * j + p_
                    dr = 64 if h % 2 == 0 else 0
                    nc.vector.tensor_copy(zun[:, h * N:(h + 1) * N], zs[p_][:, :])
                    nc.gpsimd.dma_start(  # gpsimd: casting DMA bf16 -> f32
                        out=dall[h:h + 1, :], in_=zun[dr:dr + 1, h * N:(h + 1) * N])

            def normalize(s, zun, dall):
                zt = zpool.tile([P, NDT * N], bf16, tag="zt", name=f"zt_{s}")
                rall = rpool.tile([H, N], f32, tag="rall", name=f"rall_{s}")
                nc.vector.reciprocal(rall, dall)
                rdram = drpool.tile([H, N], f32, tag="rdram", name=f"rdram_{s}")
                nc.sync.dma_start(out=rdram, in_=rall)
                for j in range(NDT):  # head pairs share one rbc tile
                    rbc = rpool.tile([P, N], f32, tag="rbc", name=f"rbc_{s}_{j}")
                    for p_ in range(2):
                        row = rdram[2 * j + p_: 2 * j + p_ + 1, :]
                        nc.sync.dma_start(
                            out=rbc[64 * p_:64 * p_ + 64, :],
                            in_=bass.AP(tensor=row.tensor, offset=row.offset,
                                        ap=[[0, 64]] + row.ap[1:]),
                        )
                    for p_ in range(2):
                        h = 2 * j + p_
                        pb = 64 * p_
                        nc.vector.tensor_tensor(
                            out=zt[pb:pb + 64, j * N:(j + 1) * N],
                            in0=zun[pb:pb + 64, h * N:(h + 1) * N],
                            in1=rbc[pb:pb + 64, :], op=Mult,
                        )
                return zt

            def out_proj_chunk(s, et, zt):
                ps = ps_s.tile([P, N], f32, tag="s", name=f"ps_o_{s}_{et}")
                for dd in range(NDT):
                    for qh in range(NQH):
                        nc.tensor.matmul(
                            ps[:, qh * 512:(qh + 1) * 512],
                            lhsT=w_sb["wo"][:, dd * 512 + et * P: dd * 512 + (et + 1) * P],
                            rhs=zt[:, dd * N + qh * 512: dd * N + qh * 512 + 512],
                            start=(dd == 0), stop=(dd == NDT - 1),
                        )
                for qh in range(NQH):
                    o_sb = opool.tile([P, 512], f32, tag="o", name=f"o_{s}_{et}_{qh}")
                    nc.vector.tensor_scalar_add(
                        o_sb, ps[:, qh * 512:(qh + 1) * 512], b_sb["bo"][:, et:et + 1])
                    nc.sync.dma_start(
                        out=OT[s, et * P:(et + 1) * P, qh * 512:(qh + 1) * 512],
                        in_=o_sb,
                    )

            # Instruction-level software pipeline: projection / out-proj
            # chunks are emitted between attention head-pairs so the PE
            # fills the idle left by the ACT-paced exp stream, and the two
            # slices' attention phases run back-to-back on ACT.
            def alloc_qk(s):
                qt = qkpool.tile([P, NDT * N], bf16, tag="qt", name=f"qt_{s}")
                kt = qkpool.tile([P, NDT * N], bf16, tag="kt", name=f"kt_{s}")
                return qt, kt

            def alloc_attn(s):
                zun = zpool.tile([P, H * N], bf16, tag="zun", name=f"zun_{s}")
                dall = rpool.tile([H, N], f32, tag="dall", name=f"dall_{s}")
                return zun, dall

            xt0 = load_x(0)
            q0, k0 = alloc_qk(0)
            proj_qk_chunk(0, 0, xt0, q0, k0)
            v0 = proj_v(0, xt0)
            zun0, dall0 = alloc_attn(0)

            attention_pair(0, 0, q0, k0, v0, zun0, dall0)
            proj_qk_chunk(0, 1, xt0, q0, k0)
            attention_pair(0, 1, q0, k0, v0, zun0, dall0)
            proj_qk_chunk(0, 2, xt0, q0, k0)
            attention_pair(0, 2, q0, k0, v0, zun0, dall0)
            proj_qk_chunk(0, 3, xt0, q0, k0)

            xt1 = load_x(1)
            q1, k1 = alloc_qk(1)
            attention_pair(0, 3, q0, k0, v0, zun0, dall0)
            proj_qk_chunk(1, 0, xt1, q1, k1)
            v1 = proj_v(1, xt1)

            zt0 = normalize(0, zun0, dall0)
            zun1, dall1 = alloc_attn(1)

            attention_pair(1, 0, q1, k1, v1, zun1, dall1)
            proj_qk_chunk(1, 1, xt1, q1, k1)
            out_proj_chunk(0, 0, zt0)
            attention_pair(1, 1, q1, k1, v1, zun1, dall1)
            proj_qk_chunk(1, 2, xt1, q1, k1)
            out_proj_chunk(0, 1, zt0)
            attention_pair(1, 2, q1, k1, v1, zun1, dall1)
            proj_qk_chunk(1, 3, xt1, q1, k1)
            out_proj_chunk(0, 2, zt0)
            attention_pair(1, 3, q1, k1, v1, zun1, dall1)
            out_proj_chunk(0, 3, zt0)

            zt1 = normalize(1, zun1, dall1)
            for et in range(NDT):
                out_proj_chunk(1, et, zt1)

    nc.compile()
    return nc


def _get_nc():
    if "nc" not in _CACHE:
        _CACHE["nc"] = _build_nc()
    return _CACHE["nc"]


def kernel(X, Wq, bq, Wk, bk, Wv, bv, Wo, bo):
    from concourse.bass_utils import run_bass_kernel_spmd

    nc = _get_nc()
    bf16 = ml_dtypes.bfloat16

    Xf = np.asarray(X, np.float32).reshape(B * T, N, D)
    XT_all = np.ascontiguousarray(Xf.transpose(0, 2, 1)).astype(bf16)  # [16, D, N]
    WQT = np.ascontiguousarray(np.asarray(Wq, np.float32).T * S_SCALE).astype(bf16)
    WKT = np.ascontiguousarray(np.asarray(Wk, np.float32).T).astype(bf16)
    WVT = np.ascontiguousarray(np.asarray(Wv, np.float32).T).astype(bf16)
    WOT = np.ascontiguousarray(np.asarray(Wo, np.float32).T).astype(bf16)
    bo_eff = (np.asarray(bo, np.float32)
              + np.asarray(Wo, np.float32) @ np.asarray(bv, np.float32))
    BQa = (np.asarray(bq, np.float32) * S_SCALE).reshape(NDT, P, 1)
    BKa = np.asarray(bk, np.float32).reshape(NDT, P, 1)
    BOa = bo_eff.reshape(NDT, P, 1)

    in_maps = []
    for c in range(NCORES):
        in_maps.append({
            "XT": np.ascontiguousarray(XT_all[c * NSLICE:(c + 1) * NSLICE]),
            "WQT": WQT, "WKT": WKT, "WVT": WVT, "WOT": WOT,
            "BQ": BQa, "BK": BKa, "BO": BOa,
        })

    trace = bool(int(os.environ.get("KERNEL_TRACE", "0")))
    kwargs = {}
    if trace:
        import tempfile
        kwargs = {"trace": True, "tmpdir": tempfile.mkdtemp(prefix="ker_trace_")}
    res = run_bass_kernel_spmd(nc, in_maps, core_ids=list(range(NCORES)), **kwargs)
    _CACHE["last_exec_ns"] = res.exec_time_ns

    out = np.empty((B * T, N, D), np.float32)
    for c in range(NCORES):
        ot = np.asarray(res.results[c]["OT"], np.float32)  # [NSLICE, D, N]
        out[c * NSLICE:(c + 1) * NSLICE] = ot.transpose(0, 2, 1)
    return out.reshape(B, T, N, D)


# revision 14
# speedup vs baseline: 1.9507x; 1.1045x over previous
"""Multi-head self-attention (AdaptiveTemporalContrastEnhancement) on 8 TRN2 cores.

Key facts baked in:
- The temporal-difference bias delta_c is added uniformly along the softmax
  axis, so softmax cancels it exactly -> it is skipped entirely.
- max |logit| ~ 1.9, so softmax runs without max-subtraction.
- V bias + output bias fold into one effective output bias:
      out = A@(XWv^T + bv) Wo^T + bo = A@(XWv^T)Wo^T + (Wo bv + bo).
- 1/sqrt(dh) is folded into WQT/BQ host-side.
- Data parallel over the 16 (b,t) slices: 2 slices per core, no collectives.
- All matmuls in bf16 (1 cyc/row on PE); accumulation fp32 in PSUM.

Device layout per slice (all "T" = dim-major, tokens along the free axis):
  XT  [d, n]   : 4 x [128, 1024] sbuf tiles (host pre-transposed)
  QT,KT [e, n] : computed as W^T.T @ XT  (4 x [128,1024])
  V_pad [n, .] : token-major, padded per head to a [128,128] stationary:
                 even head h: V cols 0-63, ones col 64, zeros 65-127
                 odd  head h: ones col 0, zeros 1-63,  V cols 64-127
                 so the PV matmul puts head h's Z^T at partitions 64*(h%2)..+63
                 and the softmax denominator at row 64 (even) / row 0 (odd).
  S^T [kv, q]  : head-PAIR packed: one [128, 1024] psum tile holds both heads'
                 S^T for one (kv, q-half); the two S matmuls use disjoint PE
                 row groups (partitions 0-63 / 64-127) and run concurrently.
  Z^T [d, q]   : per (head, q-half) [128, 512] psum accum over kv; evacuated
                 (with denominator row) to sbuf zun per head block.
  denominators : reshaped by DMA into dall8[128, 64] (head h = 16 partitions
                 x 64 cols) so ONE short-free-dim reciprocal per head PAIR is
                 cheap; broadcast back via a DRAM bounce; TT-mult per head.
  O^T [e, n]   : out-proj from normalized Z^T; host transposes back.

The schedule is software-pipelined at instruction level: projection and
out-projection chunks are emitted between attention head-pairs so the PE
fills the idle left by the ACT-paced exp stream, keeping the PE busy (and
its HAM clock-gate warm) while both slices' attention runs back-to-back.
"""

import os
import numpy as np
import ml_dtypes

B, T, N, D = 2, 8, 1024, 512
H, DH = 8, 64
P = 128
NDT = D // P          # 4 d-tiles
NKV = N // P          # 8 kv tiles
NQH = N // 512        # 2 q halves
NCORES = 8
NSLICE = (B * T) // NCORES   # 2 slices per core
S_SCALE = float(1.0 / np.sqrt(DH))  # 0.125

_CACHE = {}


def _build_nc():
    import concourse.mybir as mybir
    from concourse import bacc
    from concourse.tile import TileContext
    import concourse.bass as bass

    f32, bf16 = mybir.dt.float32, mybir.dt.bfloat16
    nc = bacc.Bacc("TRN2", target_bir_lowering=False, debug=False)

    XT = nc.dram_tensor("XT", [NSLICE, D, N], bf16, kind="ExternalInput")
    WQT = nc.dram_tensor("WQT", [D, D], bf16, kind="ExternalInput")
    WKT = nc.dram_tensor("WKT", [D, D], bf16, kind="ExternalInput")
    WVT = nc.dram_tensor("WVT", [D, D], bf16, kind="ExternalInput")
    WOT = nc.dram_tensor("WOT", [D, D], bf16, kind="ExternalInput")
    BQ = nc.dram_tensor("BQ", [NDT, P, 1], f32, kind="ExternalInput")
    BK = nc.dram_tensor("BK", [NDT, P, 1], f32, kind="ExternalInput")
    BO = nc.dram_tensor("BO", [NDT, P, 1], f32, kind="ExternalInput")
    OT = nc.dram_tensor("OT", [NSLICE, D, N], f32, kind="ExternalOutput")

    Exp = mybir.ActivationFunctionType.Exp
    Mult = mybir.AluOpType.mult

    with TileContext(nc) as tc:
        with (
            tc.tile_pool(name="wpool", bufs=1) as wpool,
            tc.tile_pool(name="xpool", bufs=2) as xpool,
            tc.tile_pool(name="qkpool", bufs=2) as qkpool,
            tc.tile_pool(name="vpool", bufs=2) as vpool,
            tc.tile_pool(name="apool", bufs=4) as apool,
            tc.tile_pool(name="zpool", bufs=2) as zpool,
            tc.tile_pool(name="rpool", bufs=2) as rpool,
            tc.tile_pool(name="opool", bufs=3) as opool,
            tc.tile_pool(name="drpool", bufs=2, space="DRAM") as drpool,
            tc.tile_pool(name="ps_s", bufs=2, space="PSUM") as ps_s,
            tc.tile_pool(name="ps_z", bufs=3, space="PSUM") as ps_z,
        ):
            # ---- persistent weights / biases (DMAs split for queue parallelism;
            #      wq/wk/bq/bk first so the first projection chunk starts early) ----
            w_sb, b_sb = {}, {}

            def emit_w(name, dram):
                t = wpool.tile([P, NDT * 512], bf16, tag=name, name=f"w_{name}")
                w_sb[name] = t
                for dt_ in range(NDT):
                    for half in range(2):
                        nc.sync.dma_start(
                            out=t[:, dt_ * 512 + half * 256: dt_ * 512 + half * 256 + 256],
                            in_=dram[dt_ * P:(dt_ + 1) * P, half * 256:half * 256 + 256],
                        )

            def emit_b(name, dram):
                t = wpool.tile([P, NDT], f32, tag=name, name=f"b_{name}")
                b_sb[name] = t
                for et in range(NDT):
                    nc.sync.dma_start(out=t[:, et:et + 1], in_=dram[et])

            def load_x(s):
                xt = xpool.tile([P, NDT * N], bf16, tag="xt", name=f"xt_{s}")
                for dt_ in range(NDT):
                    for q4 in range(4):
                        nc.sync.dma_start(
                            out=xt[:, dt_ * N + q4 * 256: dt_ * N + (q4 + 1) * 256],
                            in_=XT[s, dt_ * P:(dt_ + 1) * P, q4 * 256:(q4 + 1) * 256],
                        )
                return xt

            def proj_qk_chunk(s, et, xt, qt, kt):
                for dst, wname, bname in ((qt, "wq", "bq"), (kt, "wk", "bk")):
                    w = w_sb[wname]
                    ps = ps_s.tile([P, N], f32, tag="s", name=f"ps_{wname}_{s}_{et}")
                    for dt_ in range(NDT):
                        for qh in range(NQH):
                            nc.tensor.matmul(
                                ps[:, qh * 512:(qh + 1) * 512],
                                lhsT=w[:, dt_ * 512 + et * P: dt_ * 512 + (et + 1) * P],
                                rhs=xt[:, dt_ * N + qh * 512: dt_ * N + qh * 512 + 512],
                                start=(dt_ == 0), stop=(dt_ == NDT - 1),
                            )
                    for qh in range(NQH):
                        nc.vector.tensor_scalar_add(
                            dst[:, et * N + qh * 512: et * N + qh * 512 + 512],
                            ps[:, qh * 512:(qh + 1) * 512],
                            b_sb[bname][:, et:et + 1],
                        )

            def proj_v(s, xt):
                v_sb = vpool.tile([P, NKV * H * P], bf16, tag="v", name=f"v_{s}")
                vz = v_sb.rearrange("p (b r) -> p b r", r=256)
                nc.gpsimd.memset(vz[:, :, 65:128], 0.0)    # even-head pad
                nc.gpsimd.memset(vz[:, :, 129:192], 0.0)   # odd-head pad
                nc.vector.memset(vz[:, :, 64:65], 1.0)     # even-head ones col
                nc.vector.memset(vz[:, :, 128:129], 1.0)   # odd-head ones col
                for kv in range(NKV):
                    ps = ps_s.tile([P, N], f32, tag="s", name=f"ps_v_{s}_{kv}")
                    for dt_ in range(NDT):
                        nc.tensor.matmul(
                            ps[:, 0:512],
                            lhsT=xt[:, dt_ * N + kv * P: dt_ * N + (kv + 1) * P],
                            rhs=w_sb["wv"][:, dt_ * 512:(dt_ + 1) * 512],
                            start=(dt_ == 0), stop=(dt_ == NDT - 1),
                        )
                    vblk = v_sb[:, kv * 1024:(kv + 1) * 1024].rearrange(
                        "p (hp r) -> p hp r", r=256)
                    psh = ps[:, 0:512].rearrange("p (hp c) -> p hp c", c=128)
                    nc.vector.tensor_copy(vblk[:, :, 0:64], psh[:, :, 0:64])
                    nc.vector.tensor_copy(vblk[:, :, 192:256], psh[:, :, 64:128])
                return v_sb

            def attention_pair(s, j, qt, kt, v_sb, zun, dall8):
                """Heads 2j, 2j+1: S matmuls packed into disjoint PE row
                groups; one exp covers both heads; PV per head/q-half."""
                et = j
                for qh in range(NQH):
                    zs = [ps_z.tile([P, 512], f32, tag="z", name=f"z_{s}_{j}_{qh}_{p_}")
                          for p_ in range(2)]
                    for kv in range(NKV):
                        s_ps = ps_s.tile([P, N], f32, tag="s", name=f"s_{s}_{j}_{qh}_{kv}")
                        for p_ in range(2):
                            pb = 64 * p_
                            nc.tensor.matmul(
                                s_ps[:, p_ * 512:(p_ + 1) * 512],
                                lhsT=kt[pb:pb + 64, et * N + kv * P: et * N + (kv + 1) * P],
                                rhs=qt[pb:pb + 64, et * N + qh * 512: et * N + qh * 512 + 512],
                                start=True, stop=True,
                            )
                        at = apool.tile([P, N], bf16, tag="at", name=f"at_{s}_{j}_{qh}_{kv}")
                        nc.scalar.activation(at, s_ps, Exp)
                        for p_ in range(2):
                            h = 2 * j + p_
                            nc.tensor.matmul(
                                zs[p_],
                                lhsT=v_sb[:, kv * 1024 + h * P: kv * 1024 + (h + 1) * P],
                                rhs=at[:, p_ * 512:(p_ + 1) * 512],
                                start=(kv == 0), stop=(kv == NKV - 1),
                            )
                    for p_ in range(2):
                        h = 2 * j + p_
                        nc.vector.tensor_copy(
                            zun[:, h * N + qh * 512: h * N + qh * 512 + 512], zs[p_])
                for p_ in range(2):
                    h = 2 * j + p_
                    dr = 64 if h % 2 == 0 else 0
                    # reshape-gather the denom row into dall8[16h:16h+16, 0:64]
                    nc.gpsimd.dma_start(  # gpsimd: casting DMA bf16 -> f32
                        out=dall8[16 * h:16 * (h + 1), :],
                        in_=zun[dr:dr + 1, h * N:(h + 1) * N])

            def norm_pair(s, j, zun, dall8, rall8, rdram, zt):
                # short-free-dim reciprocal over the pair's 32 partitions
                nc.vector.reciprocal(rall8[32 * j:32 * (j + 1), :],
                                     dall8[32 * j:32 * (j + 1), :])
                nc.sync.dma_start(out=rdram[32 * j:32 * (j + 1), :],
                                  in_=rall8[32 * j:32 * (j + 1), :])
                rbc = rpool.tile([P, N], f32, tag="rbc", name=f"rbc_{s}_{j}")
                for p_ in range(2):
                    h = 2 * j + p_
                    base = rdram[0:1, 0:1]
                    nc.sync.dma_start(
                        out=rbc[64 * p_:64 * p_ + 64, :],
                        in_=bass.AP(tensor=base.tensor, offset=base.offset + h * N,
                                    ap=[[0, 64], [1, N]]),
                    )
                for p_ in range(2):
                    h = 2 * j + p_
                    pb = 64 * p_
                    nc.vector.tensor_tensor(
                        out=zt[pb:pb + 64, j * N:(j + 1) * N],
                        in0=zun[pb:pb + 64, h * N:(h + 1) * N],
                        in1=rbc[pb:pb + 64, :], op=Mult,
                    )

            def out_proj_chunk(s, et, zt):
                ps = ps_s.tile([P, N], f32, tag="s", name=f"ps_o_{s}_{et}")
                for dd in range(NDT):
                    for qh in range(NQH):
                        nc.tensor.matmul(
                            ps[:, qh * 512:(qh + 1) * 512],
                            lhsT=w_sb["wo"][:, dd * 512 + et * P: dd * 512 + (et + 1) * P],
                            rhs=zt[:, dd * N + qh * 512: dd * N + qh * 512 + 512],
                            start=(dd == 0), stop=(dd == NDT - 1),
                        )
                for qh in range(NQH):
                    o_sb = opool.tile([P, 512], f32, tag="o", name=f"o_{s}_{et}_{qh}")
                    nc.vector.tensor_scalar_add(
                        o_sb, ps[:, qh * 512:(qh + 1) * 512], b_sb["bo"][:, et:et + 1])
                    nc.sync.dma_start(
                        out=OT[s, et * P:(et + 1) * P, qh * 512:(qh + 1) * 512],
                        in_=o_sb,
                    )

            def alloc_attn(s):
                zun = zpool.tile([P, H * N], bf16, tag="zun", name=f"zun_{s}")
                zt = zpool.tile([P, NDT * N], bf16, tag="zt", name=f"zt_{s}")
                dall8 = rpool.tile([P, 64], f32, tag="dall", name=f"dall_{s}")
                rall8 = rpool.tile([P, 64], f32, tag="rall", name=f"rall_{s}")
                rdram = drpool.tile([P, 64], f32, tag="rdram", name=f"rdram_{s}")
                return zun, zt, dall8, rall8, rdram

            # ---- schedule ----
            emit_w("wq", WQT)
            emit_w("wk", WKT)
            emit_b("bq", BQ)
            emit_b("bk", BK)
            xt0 = load_x(0)
            emit_w("wv", WVT)
            emit_w("wo", WOT)
            emit_b("bo", BO)

            q0 = qkpool.tile([P, NDT * N], bf16, tag="qt", name="qt_0")
            k0 = qkpool.tile([P, NDT * N], bf16, tag="kt", name="kt_0")
            proj_qk_chunk(0, 0, xt0, q0, k0)
            v0 = proj_v(0, xt0)
            a0 = alloc_attn(0)

            attention_pair(0, 0, q0, k0, v0, a0[0], a0[2])
            proj_qk_chunk(0, 1, xt0, q0, k0)
            xt1 = load_x(1)
            norm_pair(0, 0, a0[0], a0[2], a0[3], a0[4], a0[1])

            attention_pair(0, 1, q0, k0, v0, a0[0], a0[2])
            proj_qk_chunk(0, 2, xt0, q0, k0)
            norm_pair(0, 1, a0[0], a0[2], a0[3], a0[4], a0[1])

            attention_pair(0, 2, q0, k0, v0, a0[0], a0[2])
            proj_qk_chunk(0, 3, xt0, q0, k0)
            norm_pair(0, 2, a0[0], a0[2], a0[3], a0[4], a0[1])

            q1 = qkpool.tile([P, NDT * N], bf16, tag="qt", name="qt_1")
            k1 = qkpool.tile([P, NDT * N], bf16, tag="kt", name="kt_1")
            attention_pair(0, 3, q0, k0, v0, a0[0], a0[2])
            proj_qk_chunk(1, 0, xt1, q1, k1)
            v1 = proj_v(1, xt1)
            norm_pair(0, 3, a0[0], a0[2], a0[3], a0[4], a0[1])
            a1 = alloc_attn(1)

            attention_pair(1, 0, q1, k1, v1, a1[0], a1[2])
            proj_qk_chunk(1, 1, xt1, q1, k1)
            out_proj_chunk(0, 0, a0[1])
            norm_pair(1, 0, a1[0], a1[2], a1[3], a1[4], a1[1])

            attention_pair(1, 1, q1, k1, v1, a1[0], a1[2])
            proj_qk_chunk(1, 2, xt1, q1, k1)
            out_proj_chunk(0, 1, a0[1])
            norm_pair(1, 1, a1[0], a1[2], a1[3], a1[4], a1[1])

            attention_pair(1, 2, q1, k1, v1, a1[0], a1[2])
            proj_qk_chunk(1, 3, xt1, q1, k1)
            out_proj_chunk(0, 2, a0[1])
            norm_pair(1, 2, a1[0], a1[2], a1[3], a1[4], a1[1])

            attention_pair(1, 3, q1, k1, v1, a1[0], a1[2])
            out_proj_chunk(0, 3, a0[1])
            norm_pair(1, 3, a1[0], a1[2], a1[3], a1[4], a1[1])

            for et in range(NDT):
                out_proj_chunk(1, et, a1[1])

    nc.compile()
    return nc


def _get_nc():
    if "nc" not in _CACHE:
        _CACHE["nc"] = _build_nc()
    return _CACHE["nc"]


def kernel(X, Wq, bq, Wk, bk, Wv, bv, Wo, bo):
    from concourse.bass_utils import run_bass_kernel_spmd

    nc = _get_nc()
    bf16 = ml_dtypes.bfloat16

    Xf = np.asarray(X, np.float32).reshape(B * T, N, D)
    XT_all = np.ascontiguousarray(Xf.transpose(0, 2, 1)).astype(bf16)  # [16, D, N]
    WQT = np.ascontiguousarray(np.asarray(Wq, np.float32).T * S_SCALE).astype(bf16)
    WKT = np.ascontiguousarray(np.asarray(Wk, np.float32).T).astype(bf16)
    WVT = np.ascontiguousarray(np.asarray(Wv, np.float32).T).astype(bf16)
    WOT = np.ascontiguousarray(np.asarray(Wo, np.float32).T).astype(bf16)
    bo_eff = (np.asarray(bo, np.float32)
              + np.asarray(Wo, np.float32) @ np.asarray(bv, np.float32))
    BQa = (np.asarray(bq, np.float32) * S_SCALE).reshape(NDT, P, 1)
    BKa = np.asarray(bk, np.float32).reshape(NDT, P, 1)
    BOa = bo_eff.reshape(NDT, P, 1)

    in_maps = []
    for c in range(NCORES):
        in_maps.append({
            "XT": np.ascontiguousarray(XT_all[c * NSLICE:(c + 1) * NSLICE]),
            "WQT": WQT, "WKT": WKT, "WVT": WVT, "WOT": WOT,
            "BQ": BQa, "BK": BKa, "BO": BOa,
        })

    trace = bool(int(os.environ.get("KERNEL_TRACE", "0")))
    kwargs = {}
    if trace:
        import tempfile
        kwargs = {"trace": True, "tmpdir": tempfile.mkdtemp(prefix="ker_trace_")}
    res = run_bass_kernel_spmd(nc, in_maps, core_ids=list(range(NCORES)), **kwargs)
    _CACHE["last_exec_ns"] = res.exec_time_ns

    out = np.empty((B * T, N, D), np.float32)
    for c in range(NCORES):
        ot = np.asarray(res.results[c]["OT"], np.float32)  # [NSLICE, D, N]
        out[c * NSLICE:(c + 1) * NSLICE] = ot.transpose(0, 2, 1)
    return out.reshape(B, T, N, D)


# revision 15
# speedup vs baseline: 2.0106x; 1.0307x over previous
"""Multi-head self-attention (AdaptiveTemporalContrastEnhancement) on 8 TRN2 cores.

Key facts baked in:
- The temporal-difference bias delta_c is added uniformly along the softmax
  axis, so softmax cancels it exactly -> it is skipped entirely.
- max |logit| ~ 1.9, so softmax runs without max-subtraction.
- V bias + output bias fold into one effective output bias:
      out = A@(XWv^T + bv) Wo^T + bo = A@(XWv^T)Wo^T + (Wo bv + bo).
- 1/sqrt(dh) is folded into WQT/BQ host-side.
- Data parallel over the 16 (b,t) slices: 2 slices per core, no collectives.
- All matmuls in bf16 (1 cyc/row on PE); accumulation fp32 in PSUM.

Device layout per slice (all "T" = dim-major, tokens along the free axis):
  XT  [d, n]   : 4 x [128, 1024] sbuf tiles (host pre-transposed)
  QT,KT [e, n] : computed as W^T.T @ XT  (4 x [128,1024])
  V_pad [n, .] : token-major, padded per head to a [128,128] stationary:
                 even head h: V cols 0-63, ones col 64, zeros 65-127
                 odd  head h: ones col 0, zeros 1-63,  V cols 64-127
                 so the PV matmul puts head h's Z^T at partitions 64*(h%2)..+63
                 and the softmax denominator at row 64 (even) / row 0 (odd).
  S^T [kv, q]  : head-PAIR packed: one [128, 1024] psum tile holds both heads'
                 S^T for one (kv, q-half); the two S matmuls use disjoint PE
                 row groups (partitions 0-63 / 64-127) and run concurrently.
  Z^T [d, q]   : per (head, q-half) [128, 512] psum accum over kv; evacuated
                 (with denominator row) to sbuf zun per head block.
  denominators : reshaped by DMA into dall8[128, 64] (head h = 16 partitions
                 x 64 cols) so ONE short-free-dim reciprocal per head PAIR is
                 cheap; broadcast back via a DRAM bounce; TT-mult per head.
  O^T [e, n]   : out-proj from normalized Z^T; host transposes back.

The schedule is software-pipelined at instruction level: projection and
out-projection chunks are emitted between attention head-pairs so the PE
fills the idle left by the ACT-paced exp stream, keeping the PE busy (and
its HAM clock-gate warm) while both slices' attention runs back-to-back.
"""

import os
import numpy as np
import ml_dtypes

B, T, N, D = 2, 8, 1024, 512
H, DH = 8, 64
P = 128
NDT = D // P          # 4 d-tiles
NKV = N // P          # 8 kv tiles
NQH = N // 512        # 2 q halves
NCORES = 8
NSLICE = (B * T) // NCORES   # 2 slices per core
S_SCALE = float(1.0 / np.sqrt(DH))  # 0.125

_CACHE = {}


def _build_nc():
    import concourse.mybir as mybir
    from concourse import bacc
    from concourse.tile import TileContext
    import concourse.bass as bass

    f32, bf16 = mybir.dt.float32, mybir.dt.bfloat16
    nc = bacc.Bacc("TRN2", target_bir_lowering=False, debug=False)

    XT = nc.dram_tensor("XT", [NSLICE, D, N], bf16, kind="ExternalInput")
    WQT = nc.dram_tensor("WQT", [D, D], bf16, kind="ExternalInput")
    WKT = nc.dram_tensor("WKT", [D, D], bf16, kind="ExternalInput")
    WVT = nc.dram_tensor("WVT", [D, D], bf16, kind="ExternalInput")
    WOT = nc.dram_tensor("WOT", [D, D], bf16, kind="ExternalInput")
    BQ = nc.dram_tensor("BQ", [NDT, P, 1], f32, kind="ExternalInput")
    BK = nc.dram_tensor("BK", [NDT, P, 1], f32, kind="ExternalInput")
    BO = nc.dram_tensor("BO", [NDT, P, 1], f32, kind="ExternalInput")
    OT = nc.dram_tensor("OT", [NSLICE, D, N], f32, kind="ExternalOutput")

    Exp = mybir.ActivationFunctionType.Exp
    Mult = mybir.AluOpType.mult

    with TileContext(nc) as tc:
        with (
            tc.tile_pool(name="wpool", bufs=1) as wpool,
            tc.tile_pool(name="xpool", bufs=2) as xpool,
            tc.tile_pool(name="qkpool", bufs=2) as qkpool,
            tc.tile_pool(name="vpool", bufs=2) as vpool,
            tc.tile_pool(name="apool", bufs=4) as apool,
            tc.tile_pool(name="zpool", bufs=2) as zpool,
            tc.tile_pool(name="rpool", bufs=2) as rpool,
            tc.tile_pool(name="opool", bufs=3) as opool,
            tc.tile_pool(name="drpool", bufs=2, space="DRAM") as drpool,
            tc.tile_pool(name="ps_s", bufs=2, space="PSUM") as ps_s,
            tc.tile_pool(name="ps_z", bufs=4, space="PSUM") as ps_z,
        ):
            # ---- persistent weights / biases (DMAs split for queue parallelism;
            #      wq/wk/bq/bk first so the first projection chunk starts early) ----
            w_sb, b_sb = {}, {}

            def emit_w(name, dram):
                # one 3D-AP DMA per weight: [512,512] dram -> [128, 4*512] sbuf
                t = wpool.tile([P, NDT * 512], bf16, tag=name, name=f"w_{name}")
                w_sb[name] = t
                nc.sync.dma_start(
                    out=t[:, :].rearrange("p (dt e) -> p dt e", e=512),
                    in_=dram[:, :].rearrange("(dt p) e -> p dt e", p=P),
                )

            def emit_b(name, dram):
                t = wpool.tile([P, NDT], f32, tag=name, name=f"b_{name}")
                b_sb[name] = t
                nc.sync.dma_start(
                    out=t[:, :],
                    in_=dram[:, :, :].rearrange("et p one -> p (et one)"),
                )

            def load_x(s):
                # one 1MB DMA: large transfers reach full fabric bandwidth
                xt = xpool.tile([P, NDT * N], bf16, tag="xt", name=f"xt_{s}")
                nc.sync.dma_start(
                    out=xt[:, :].rearrange("p (dt n) -> p dt n", n=N),
                    in_=XT[s].rearrange("(dt p) n -> p dt n", p=P),
                )
                return xt

            def proj_qk_chunk(s, et, xt, qt, kt):
                for dst, wname, bname in ((qt, "wq", "bq"), (kt, "wk", "bk")):
                    w = w_sb[wname]
                    ps = ps_s.tile([P, N], f32, tag="s", name=f"ps_{wname}_{s}_{et}")
                    for dt_ in range(NDT):
                        for qh in range(NQH):
                            nc.tensor.matmul(
                                ps[:, qh * 512:(qh + 1) * 512],
                                lhsT=w[:, dt_ * 512 + et * P: dt_ * 512 + (et + 1) * P],
                                rhs=xt[:, dt_ * N + qh * 512: dt_ * N + qh * 512 + 512],
                                start=(dt_ == 0), stop=(dt_ == NDT - 1),
                            )
                    for qh in range(NQH):
                        nc.vector.tensor_scalar_add(
                            dst[:, et * N + qh * 512: et * N + qh * 512 + 512],
                            ps[:, qh * 512:(qh + 1) * 512],
                            b_sb[bname][:, et:et + 1],
                        )

            def proj_v(s, xt):
                v_sb = vpool.tile([P, NKV * H * P], bf16, tag="v", name=f"v_{s}")
                vz = v_sb.rearrange("p (b r) -> p b r", r=256)
                nc.gpsimd.memset(vz[:, :, 65:128], 0.0)    # even-head pad
                nc.gpsimd.memset(vz[:, :, 129:192], 0.0)   # odd-head pad
                nc.vector.memset(vz[:, :, 64:65], 1.0)     # even-head ones col
                nc.vector.memset(vz[:, :, 128:129], 1.0)   # odd-head ones col
                for kv in range(NKV):
                    ps = ps_s.tile([P, N], f32, tag="s", name=f"ps_v_{s}_{kv}")
                    for dt_ in range(NDT):
                        nc.tensor.matmul(
                            ps[:, 0:512],
                            lhsT=xt[:, dt_ * N + kv * P: dt_ * N + (kv + 1) * P],
                            rhs=w_sb["wv"][:, dt_ * 512:(dt_ + 1) * 512],
                            start=(dt_ == 0), stop=(dt_ == NDT - 1),
                        )
                    vblk = v_sb[:, kv * 1024:(kv + 1) * 1024].rearrange(
                        "p (hp r) -> p hp r", r=256)
                    psh = ps[:, 0:512].rearrange("p (hp c) -> p hp c", c=128)
                    nc.vector.tensor_copy(vblk[:, :, 0:64], psh[:, :, 0:64])
                    nc.vector.tensor_copy(vblk[:, :, 192:256], psh[:, :, 64:128])
                return v_sb

            def attention_pair(s, j, qt, kt, v_sb, zun, dall8):
                """Heads 2j, 2j+1: S matmuls packed into disjoint PE row
                groups; one exp covers both heads; PV per head/q-half."""
                et = j
                for qh in range(NQH):
                    zs = [ps_z.tile([P, 512], f32, tag="z", name=f"z_{s}_{j}_{qh}_{p_}")
                          for p_ in range(2)]
                    for kv in range(NKV):
                        s_ps = ps_s.tile([P, N], f32, tag="s", name=f"s_{s}_{j}_{qh}_{kv}")
                        for p_ in range(2):
                            pb = 64 * p_
                            nc.tensor.matmul(
                                s_ps[:, p_ * 512:(p_ + 1) * 512],
                                lhsT=kt[pb:pb + 64, et * N + kv * P: et * N + (kv + 1) * P],
                                rhs=qt[pb:pb + 64, et * N + qh * 512: et * N + qh * 512 + 512],
                                start=True, stop=True,
                            )
                        at = apool.tile([P, N], bf16, tag="at", name=f"at_{s}_{j}_{qh}_{kv}")
                        nc.scalar.activation(at, s_ps, Exp)
                        for p_ in range(2):
                            h = 2 * j + p_
                            nc.tensor.matmul(
                                zs[p_],
                                lhsT=v_sb[:, kv * 1024 + h * P: kv * 1024 + (h + 1) * P],
                                rhs=at[:, p_ * 512:(p_ + 1) * 512],
                                start=(kv == 0), stop=(kv == NKV - 1),
                            )
                    for p_ in range(2):
                        h = 2 * j + p_
                        nc.vector.tensor_copy(
                            zun[:, h * N + qh * 512: h * N + qh * 512 + 512], zs[p_])
                for p_ in range(2):
                    h = 2 * j + p_
                    dr = 64 if h % 2 == 0 else 0
                    # reshape-gather the denom row into dall8[16h:16h+16, 0:64]
                    nc.gpsimd.dma_start(  # gpsimd: casting DMA bf16 -> f32
                        out=dall8[16 * h:16 * (h + 1), :],
                        in_=zun[dr:dr + 1, h * N:(h + 1) * N])

            def norm_pair(s, j, zun, dall8, rall8, rdram, zt):
                # short-free-dim reciprocal over the pair's 32 partitions
                nc.vector.reciprocal(rall8[32 * j:32 * (j + 1), :],
                                     dall8[32 * j:32 * (j + 1), :])
                nc.sync.dma_start(out=rdram[32 * j:32 * (j + 1), :],
                                  in_=rall8[32 * j:32 * (j + 1), :])
                rbc = rpool.tile([P, N], f32, tag="rbc", name=f"rbc_{s}_{j}")
                for p_ in range(2):
                    h = 2 * j + p_
                    base = rdram[0:1, 0:1]
                    nc.sync.dma_start(
                        out=rbc[64 * p_:64 * p_ + 64, :],
                        in_=bass.AP(tensor=base.tensor, offset=base.offset + h * N,
                                    ap=[[0, 64], [1, N]]),
                    )
                for p_ in range(2):
                    h = 2 * j + p_
                    pb = 64 * p_
                    nc.vector.tensor_tensor(
                        out=zt[pb:pb + 64, j * N:(j + 1) * N],
                        in0=zun[pb:pb + 64, h * N:(h + 1) * N],
                        in1=rbc[pb:pb + 64, :], op=Mult,
                    )

            def out_proj_chunk(s, et, zt):
                ps = ps_s.tile([P, N], f32, tag="s", name=f"ps_o_{s}_{et}")
                for dd in range(NDT):
                    for qh in range(NQH):
                        nc.tensor.matmul(
                            ps[:, qh * 512:(qh + 1) * 512],
                            lhsT=w_sb["wo"][:, dd * 512 + et * P: dd * 512 + (et + 1) * P],
                            rhs=zt[:, dd * N + qh * 512: dd * N + qh * 512 + 512],
                            start=(dd == 0), stop=(dd == NDT - 1),
                        )
                o_sb = opool.tile([P, N], f32, tag="o", name=f"o_{s}_{et}")
                for qh in range(NQH):
                    nc.vector.tensor_scalar_add(
                        o_sb[:, qh * 512:(qh + 1) * 512],
                        ps[:, qh * 512:(qh + 1) * 512], b_sb["bo"][:, et:et + 1])
                nc.sync.dma_start(out=OT[s, et * P:(et + 1) * P, :], in_=o_sb)

            def alloc_attn(s):
                zun = zpool.tile([P, H * N], bf16, tag="zun", name=f"zun_{s}")
                zt = zpool.tile([P, NDT * N], bf16, tag="zt", name=f"zt_{s}")
                dall8 = rpool.tile([P, 64], f32, tag="dall", name=f"dall_{s}")
                rall8 = rpool.tile([P, 64], f32, tag="rall", name=f"rall_{s}")
                rdram = drpool.tile([P, 64], f32, tag="rdram", name=f"rdram_{s}")
                return zun, zt, dall8, rall8, rdram

            # ---- schedule ----
            emit_w("wq", WQT)
            emit_w("wk", WKT)
            emit_b("bq", BQ)
            emit_b("bk", BK)
            xt0 = load_x(0)
            emit_w("wv", WVT)
            emit_w("wo", WOT)
            emit_b("bo", BO)

            q0 = qkpool.tile([P, NDT * N], bf16, tag="qt", name="qt_0")
            k0 = qkpool.tile([P, NDT * N], bf16, tag="kt", name="kt_0")
            proj_qk_chunk(0, 0, xt0, q0, k0)
            v0 = proj_v(0, xt0)
            a0 = alloc_attn(0)

            attention_pair(0, 0, q0, k0, v0, a0[0], a0[2])
            proj_qk_chunk(0, 1, xt0, q0, k0)
            xt1 = load_x(1)
            norm_pair(0, 0, a0[0], a0[2], a0[3], a0[4], a0[1])

            attention_pair(0, 1, q0, k0, v0, a0[0], a0[2])
            proj_qk_chunk(0, 2, xt0, q0, k0)
            norm_pair(0, 1, a0[0], a0[2], a0[3], a0[4], a0[1])

            attention_pair(0, 2, q0, k0, v0, a0[0], a0[2])
            proj_qk_chunk(0, 3, xt0, q0, k0)
            norm_pair(0, 2, a0[0], a0[2], a0[3], a0[4], a0[1])

            q1 = qkpool.tile([P, NDT * N], bf16, tag="qt", name="qt_1")
            k1 = qkpool.tile([P, NDT * N], bf16, tag="kt", name="kt_1")
            attention_pair(0, 3, q0, k0, v0, a0[0], a0[2])
            proj_qk_chunk(1, 0, xt1, q1, k1)
            v1 = proj_v(1, xt1)
            norm_pair(0, 3, a0[0], a0[2], a0[3], a0[4], a0[1])
            a1 = alloc_attn(1)

            attention_pair(1, 0, q1, k1, v1, a1[0], a1[2])
            proj_qk_chunk(1, 1, xt1, q1, k1)
            out_proj_chunk(0, 0, a0[1])
            norm_pair(1, 0, a1[0], a1[2], a1[3], a1[4], a1[1])

            attention_pair(1, 1, q1, k1, v1, a1[0], a1[2])
            proj_qk_chunk(1, 2, xt1, q1, k1)
            out_proj_chunk(0, 1, a0[1])
            norm_pair(1, 1, a1[0], a1[2], a1[3], a1[4], a1[1])

            attention_pair(1, 2, q1, k1, v1, a1[0], a1[2])
            proj_qk_chunk(1, 3, xt1, q1, k1)
            out_proj_chunk(0, 2, a0[1])
            norm_pair(1, 2, a1[0], a1[2], a1[3], a1[4], a1[1])

            attention_pair(1, 3, q1, k1, v1, a1[0], a1[2])
            out_proj_chunk(0, 3, a0[1])
            norm_pair(1, 3, a1[0], a1[2], a1[3], a1[4], a1[1])

            for et in range(NDT):
                out_proj_chunk(1, et, a1[1])

    nc.compile()
    return nc


def _get_nc():
    if "nc" not in _CACHE:
        _CACHE["nc"] = _build_nc()
    return _CACHE["nc"]


def kernel(X, Wq, bq, Wk, bk, Wv, bv, Wo, bo):
    from concourse.bass_utils import run_bass_kernel_spmd

    nc = _get_nc()
    bf16 = ml_dtypes.bfloat16

    Xf = np.asarray(X, np.float32).reshape(B * T, N, D)
    XT_all = np.ascontiguousarray(Xf.transpose(0, 2, 1)).astype(bf16)  # [16, D, N]
    WQT = np.ascontiguousarray(np.asarray(Wq, np.float32).T * S_SCALE).astype(bf16)
    WKT = np.ascontiguousarray(np.asarray(Wk, np.float32).T).astype(bf16)
    WVT = np.ascontiguousarray(np.asarray(Wv, np.float32).T).astype(bf16)
    WOT = np.ascontiguousarray(np.asarray(Wo, np.float32).T).astype(bf16)
    bo_eff = (np.asarray(bo, np.float32)
              + np.asarray(Wo, np.float32) @ np.asarray(bv, np.float32))
    BQa = (np.asarray(bq, np.float32) * S_SCALE).reshape(NDT, P, 1)
    BKa = np.asarray(bk, np.float32).reshape(NDT, P, 1)
    BOa = bo_eff.reshape(NDT, P, 1)

    in_maps = []
    for c in range(NCORES):
        in_maps.append({
            "XT": np.ascontiguousarray(XT_all[c * NSLICE:(c + 1) * NSLICE]),
            "WQT": WQT, "WKT": WKT, "WVT": WVT, "WOT": WOT,
            "BQ": BQa, "BK": BKa, "BO": BOa,
        })

    trace = bool(int(os.environ.get("KERNEL_TRACE", "0")))
    kwargs = {}
    if trace:
        import tempfile
        kwargs = {"trace": True, "tmpdir": tempfile.mkdtemp(prefix="ker_trace_")}
    res = run_bass_kernel_spmd(nc, in_maps, core_ids=list(range(NCORES)), **kwargs)
    _CACHE["last_exec_ns"] = res.exec_time_ns

    out = np.empty((B * T, N, D), np.float32)
    for c in range(NCORES):
        ot = np.asarray(res.results[c]["OT"], np.float32)  # [NSLICE, D, N]
        out[c * NSLICE:(c + 1) * NSLICE] = ot.transpose(0, 2, 1)
    return out.reshape(B, T, N, D)


# revision 16
# speedup vs baseline: 2.0488x; 1.0190x over previous
"""Multi-head self-attention (AdaptiveTemporalContrastEnhancement) on 8 TRN2 cores.

Key facts baked in:
- The temporal-difference bias delta_c is added uniformly along the softmax
  axis, so softmax cancels it exactly -> it is skipped entirely.
- max |logit| ~ 1.9, so softmax runs without max-subtraction.
- V bias + output bias fold into one effective output bias:
      out = A@(XWv^T + bv) Wo^T + bo = A@(XWv^T)Wo^T + (Wo bv + bo).
- 1/sqrt(dh) is folded into WQT/BQ host-side.
- Data parallel over the 16 (b,t) slices: 2 slices per core, no collectives.
- All matmuls in bf16 (1 cyc/row on PE); accumulation fp32 in PSUM.

Device layout per slice (all "T" = dim-major, tokens along the free axis):
  XT  [d, n]   : 4 x [128, 1024] sbuf tiles (host pre-transposed)
  QT,KT [e, n] : computed as W^T.T @ XT  (4 x [128,1024])
  V_pad [n, .] : token-major, padded per head to a [128,128] stationary:
                 even head h: V cols 0-63, ones col 64, zeros 65-127
                 odd  head h: ones col 0, zeros 1-63,  V cols 64-127
                 so the PV matmul puts head h's Z^T at partitions 64*(h%2)..+63
                 and the softmax denominator at row 64 (even) / row 0 (odd).
  S^T [kv, q]  : head-PAIR packed: one [128, 1024] psum tile holds both heads'
                 S^T for one (kv, q-half); the two S matmuls use disjoint PE
                 row groups (partitions 0-63 / 64-127) and run concurrently.
  Z^T [d, q]   : per (head, q-half) [128, 512] psum accum over kv; evacuated
                 (with denominator row) to sbuf zun per head block.
  denominators : reshaped by DMA into dall8[128, 64] (head h = 16 partitions
                 x 64 cols) so ONE short-free-dim reciprocal per head PAIR is
                 cheap; broadcast back via a DRAM bounce; TT-mult per head.
  O^T [e, n]   : out-proj from normalized Z^T; host transposes back.

The schedule is software-pipelined at instruction level: projection and
out-projection chunks are emitted between attention head-pairs so the PE
fills the idle left by the ACT-paced exp stream, keeping the PE busy (and
its HAM clock-gate warm) while both slices' attention runs back-to-back.
"""

import os
import numpy as np
import ml_dtypes

B, T, N, D = 2, 8, 1024, 512
H, DH = 8, 64
P = 128
NDT = D // P          # 4 d-tiles
NKV = N // P          # 8 kv tiles
NQH = N // 512        # 2 q halves
NCORES = 8
NSLICE = (B * T) // NCORES   # 2 slices per core
S_SCALE = float(1.0 / np.sqrt(DH))  # 0.125

_CACHE = {}


def _build_nc():
    import concourse.mybir as mybir
    from concourse import bacc
    from concourse.tile import TileContext
    import concourse.bass as bass

    f32, bf16 = mybir.dt.float32, mybir.dt.bfloat16
    nc = bacc.Bacc("TRN2", target_bir_lowering=False, debug=False)

    XT = nc.dram_tensor("XT", [NSLICE, D, N], bf16, kind="ExternalInput")
    WQT = nc.dram_tensor("WQT", [D, D], bf16, kind="ExternalInput")
    WKT = nc.dram_tensor("WKT", [D, D], bf16, kind="ExternalInput")
    WVT = nc.dram_tensor("WVT", [D, D], bf16, kind="ExternalInput")
    WOT = nc.dram_tensor("WOT", [D, D], bf16, kind="ExternalInput")
    BQ = nc.dram_tensor("BQ", [NDT, P, 1], f32, kind="ExternalInput")
    BK = nc.dram_tensor("BK", [NDT, P, 1], f32, kind="ExternalInput")
    BO = nc.dram_tensor("BO", [NDT, P, 1], f32, kind="ExternalInput")
    OT = nc.dram_tensor("OT", [NSLICE, D, N], f32, kind="ExternalOutput")

    Exp = mybir.ActivationFunctionType.Exp
    Mult = mybir.AluOpType.mult

    with TileContext(nc) as tc:
        with (
            tc.tile_pool(name="wpool", bufs=1) as wpool,
            tc.tile_pool(name="xpool", bufs=2) as xpool,
            tc.tile_pool(name="qkpool", bufs=2) as qkpool,
            tc.tile_pool(name="vpool", bufs=2) as vpool,
            tc.tile_pool(name="apool", bufs=4) as apool,
            tc.tile_pool(name="zpool", bufs=2) as zpool,
            tc.tile_pool(name="rpool", bufs=2) as rpool,
            tc.tile_pool(name="opool", bufs=3) as opool,
            tc.tile_pool(name="drpool", bufs=2, space="DRAM") as drpool,
            tc.tile_pool(name="ps_s", bufs=2, space="PSUM") as ps_s,
            tc.tile_pool(name="ps_z", bufs=4, space="PSUM") as ps_z,
        ):
            # ---- persistent weights / biases (DMAs split for queue parallelism;
            #      wq/wk/bq/bk first so the first projection chunk starts early) ----
            w_sb, b_sb = {}, {}

            def emit_w(name, dram):
                # one 3D-AP DMA per weight: [512,512] dram -> [128, 4*512] sbuf
                t = wpool.tile([P, NDT * 512], bf16, tag=name, name=f"w_{name}")
                w_sb[name] = t
                nc.sync.dma_start(
                    out=t[:, :].rearrange("p (dt e) -> p dt e", e=512),
                    in_=dram[:, :].rearrange("(dt p) e -> p dt e", p=P),
                )

            def emit_b(name, dram):
                t = wpool.tile([P, NDT], f32, tag=name, name=f"b_{name}")
                b_sb[name] = t
                nc.sync.dma_start(
                    out=t[:, :],
                    in_=dram[:, :, :].rearrange("et p one -> p (et one)"),
                )

            def load_x(s):
                # one 1MB DMA: large transfers reach full fabric bandwidth
                xt = xpool.tile([P, NDT * N], bf16, tag="xt", name=f"xt_{s}")
                nc.sync.dma_start(
                    out=xt[:, :].rearrange("p (dt n) -> p dt n", n=N),
                    in_=XT[s].rearrange("(dt p) n -> p dt n", p=P),
                )
                return xt

            def proj_qk_chunk(s, et, xt, qt, kt):
                for dst, wname, bname in ((qt, "wq", "bq"), (kt, "wk", "bk")):
                    w = w_sb[wname]
                    ps = ps_s.tile([P, N], f32, tag="s", name=f"ps_{wname}_{s}_{et}")
                    for dt_ in range(NDT):
                        for qh in range(NQH):
                            nc.tensor.matmul(
                                ps[:, qh * 512:(qh + 1) * 512],
                                lhsT=w[:, dt_ * 512 + et * P: dt_ * 512 + (et + 1) * P],
                                rhs=xt[:, dt_ * N + qh * 512: dt_ * N + qh * 512 + 512],
                                start=(dt_ == 0), stop=(dt_ == NDT - 1),
                            )
                    for qh in range(NQH):
                        nc.vector.tensor_scalar_add(
                            dst[:, et * N + qh * 512: et * N + qh * 512 + 512],
                            ps[:, qh * 512:(qh + 1) * 512],
                            b_sb[bname][:, et:et + 1],
                        )

            def proj_v(s, xt):
                v_sb = vpool.tile([P, NKV * H * P], bf16, tag="v", name=f"v_{s}")
                vz = v_sb.rearrange("p (b r) -> p b r", r=256)
                nc.gpsimd.memset(vz[:, :, 65:128], 0.0)    # even-head pad
                nc.gpsimd.memset(vz[:, :, 129:192], 0.0)   # odd-head pad
                nc.vector.memset(vz[:, :, 64:65], 1.0)     # even-head ones col
                nc.vector.memset(vz[:, :, 128:129], 1.0)   # odd-head ones col
                for kv in range(NKV):
                    ps = ps_s.tile([P, N], f32, tag="s", name=f"ps_v_{s}_{kv}")
                    for dt_ in range(NDT):
                        nc.tensor.matmul(
                            ps[:, 0:512],
                            lhsT=xt[:, dt_ * N + kv * P: dt_ * N + (kv + 1) * P],
                            rhs=w_sb["wv"][:, dt_ * 512:(dt_ + 1) * 512],
                            start=(dt_ == 0), stop=(dt_ == NDT - 1),
                        )
                    vblk = v_sb[:, kv * 1024:(kv + 1) * 1024].rearrange(
                        "p (hp r) -> p hp r", r=256)
                    psh = ps[:, 0:512].rearrange("p (hp c) -> p hp c", c=128)
                    nc.vector.tensor_copy(vblk[:, :, 0:64], psh[:, :, 0:64])
                    nc.vector.tensor_copy(vblk[:, :, 192:256], psh[:, :, 64:128])
                return v_sb

            def attention_pair(s, j, qt, kt, v_sb, zun, dall8):
                """Heads 2j, 2j+1: S matmuls packed into disjoint PE row
                groups; one exp covers both heads; PV per head/q-half."""
                et = j
                for qh in range(NQH):
                    zs = [ps_z.tile([P, 512], f32, tag="z", name=f"z_{s}_{j}_{qh}_{p_}")
                          for p_ in range(2)]
                    for kv in range(NKV):
                        s_ps = ps_s.tile([P, N], f32, tag="s", name=f"s_{s}_{j}_{qh}_{kv}")
                        for p_ in range(2):
                            pb = 64 * p_
                            nc.tensor.matmul(
                                s_ps[:, p_ * 512:(p_ + 1) * 512],
                                lhsT=kt[pb:pb + 64, et * N + kv * P: et * N + (kv + 1) * P],
                                rhs=qt[pb:pb + 64, et * N + qh * 512: et * N + qh * 512 + 512],
                                start=True, stop=True,
                            )
                        at = apool.tile([P, N], bf16, tag="at", name=f"at_{s}_{j}_{qh}_{kv}")
                        nc.scalar.activation(at, s_ps, Exp)
                        for p_ in range(2):
                            h = 2 * j + p_
                            nc.tensor.matmul(
                                zs[p_],
                                lhsT=v_sb[:, kv * 1024 + h * P: kv * 1024 + (h + 1) * P],
                                rhs=at[:, p_ * 512:(p_ + 1) * 512],
                                start=(kv == 0), stop=(kv == NKV - 1),
                            )
                    for p_ in range(2):
                        h = 2 * j + p_
                        nc.vector.tensor_copy(
                            zun[:, h * N + qh * 512: h * N + qh * 512 + 512], zs[p_])
                for p_ in range(2):
                    h = 2 * j + p_
                    dr = 64 if h % 2 == 0 else 0
                    # reshape-gather the denom row into dall8[16h:16h+16, 0:64]
                    nc.gpsimd.dma_start(  # gpsimd: casting DMA bf16 -> f32
                        out=dall8[16 * h:16 * (h + 1), :],
                        in_=zun[dr:dr + 1, h * N:(h + 1) * N])

            def norm_pair(s, j, zun, dall8, rall8, rdram, zt):
                # short-free-dim reciprocal over the pair's 32 partitions
                nc.vector.reciprocal(rall8[32 * j:32 * (j + 1), :],
                                     dall8[32 * j:32 * (j + 1), :])
                nc.sync.dma_start(out=rdram[32 * j:32 * (j + 1), :],
                                  in_=rall8[32 * j:32 * (j + 1), :])
                rbc = rpool.tile([P, N], f32, tag="rbc", name=f"rbc_{s}_{j}")
                for p_ in range(2):
                    h = 2 * j + p_
                    base = rdram[0:1, 0:1]
                    nc.sync.dma_start(
                        out=rbc[64 * p_:64 * p_ + 64, :],
                        in_=bass.AP(tensor=base.tensor, offset=base.offset + h * N,
                                    ap=[[0, 64], [1, N]]),
                    )
                for p_ in range(2):
                    h = 2 * j + p_
                    pb = 64 * p_
                    nc.vector.tensor_tensor(
                        out=zt[pb:pb + 64, j * N:(j + 1) * N],
                        in0=zun[pb:pb + 64, h * N:(h + 1) * N],
                        in1=rbc[pb:pb + 64, :], op=Mult,
                    )

            def out_proj_tail(s, ets, zt):
                """Out-proj for two e-tiles with the dd=3 (last head pair)
                contraction deferred, so these matmuls start before the last
                norm_pair's TT-mults have produced zt block 3."""
                pss = {}
                for et in ets:
                    ps = ps_s.tile([P, N], f32, tag="s", name=f"ps_ot_{s}_{et}")
                    pss[et] = ps
                    for dd in range(NDT - 1):
                        for qh in range(NQH):
                            nc.tensor.matmul(
                                ps[:, qh * 512:(qh + 1) * 512],
                                lhsT=w_sb["wo"][:, dd * 512 + et * P: dd * 512 + (et + 1) * P],
                                rhs=zt[:, dd * N + qh * 512: dd * N + qh * 512 + 512],
                                start=(dd == 0), stop=False,
                            )
                for et in ets:
                    ps = pss[et]
                    dd = NDT - 1
                    for qh in range(NQH):
                        nc.tensor.matmul(
                            ps[:, qh * 512:(qh + 1) * 512],
                            lhsT=w_sb["wo"][:, dd * 512 + et * P: dd * 512 + (et + 1) * P],
                            rhs=zt[:, dd * N + qh * 512: dd * N + qh * 512 + 512],
                            start=False, stop=True,
                        )
                    o_sb = opool.tile([P, N], f32, tag="o", name=f"o_{s}_{et}")
                    for qh in range(NQH):
                        nc.vector.tensor_scalar_add(
                            o_sb[:, qh * 512:(qh + 1) * 512],
                            ps[:, qh * 512:(qh + 1) * 512], b_sb["bo"][:, et:et + 1])
                    nc.sync.dma_start(out=OT[s, et * P:(et + 1) * P, :], in_=o_sb)

            def out_proj_chunk(s, et, zt):
                ps = ps_s.tile([P, N], f32, tag="s", name=f"ps_o_{s}_{et}")
                for dd in range(NDT):
                    for qh in range(NQH):
                        nc.tensor.matmul(
                            ps[:, qh * 512:(qh + 1) * 512],
                            lhsT=w_sb["wo"][:, dd * 512 + et * P: dd * 512 + (et + 1) * P],
                            rhs=zt[:, dd * N + qh * 512: dd * N + qh * 512 + 512],
                            start=(dd == 0), stop=(dd == NDT - 1),
                        )
                o_sb = opool.tile([P, N], f32, tag="o", name=f"o_{s}_{et}")
                for qh in range(NQH):
                    nc.vector.tensor_scalar_add(
                        o_sb[:, qh * 512:(qh + 1) * 512],
                        ps[:, qh * 512:(qh + 1) * 512], b_sb["bo"][:, et:et + 1])
                nc.sync.dma_start(out=OT[s, et * P:(et + 1) * P, :], in_=o_sb)

            def alloc_attn(s):
                zun = zpool.tile([P, H * N], bf16, tag="zun", name=f"zun_{s}")
                zt = zpool.tile([P, NDT * N], bf16, tag="zt", name=f"zt_{s}")
                dall8 = rpool.tile([P, 64], f32, tag="dall", name=f"dall_{s}")
                rall8 = rpool.tile([P, 64], f32, tag="rall", name=f"rall_{s}")
                rdram = drpool.tile([P, 64], f32, tag="rdram", name=f"rdram_{s}")
                return zun, zt, dall8, rall8, rdram

            # ---- schedule ----
            emit_w("wq", WQT)
            emit_w("wk", WKT)
            emit_b("bq", BQ)
            emit_b("bk", BK)
            xt0 = load_x(0)
            emit_w("wv", WVT)
            emit_w("wo", WOT)
            emit_b("bo", BO)

            q0 = qkpool.tile([P, NDT * N], bf16, tag="qt", name="qt_0")
            k0 = qkpool.tile([P, NDT * N], bf16, tag="kt", name="kt_0")
            proj_qk_chunk(0, 0, xt0, q0, k0)
            v0 = proj_v(0, xt0)
            a0 = alloc_attn(0)
            q1 = qkpool.tile([P, NDT * N], bf16, tag="qt", name="qt_1")
            k1 = qkpool.tile([P, NDT * N], bf16, tag="kt", name="kt_1")

            attention_pair(0, 0, q0, k0, v0, a0[0], a0[2])
            proj_qk_chunk(0, 1, xt0, q0, k0)
            norm_pair(0, 0, a0[0], a0[2], a0[3], a0[4], a0[1])

            attention_pair(0, 1, q0, k0, v0, a0[0], a0[2])
            proj_qk_chunk(0, 2, xt0, q0, k0)
            xt1 = load_x(1)
            norm_pair(0, 1, a0[0], a0[2], a0[3], a0[4], a0[1])

            attention_pair(0, 2, q0, k0, v0, a0[0], a0[2])
            proj_qk_chunk(0, 3, xt0, q0, k0)
            proj_qk_chunk(1, 0, xt1, q1, k1)
            norm_pair(0, 2, a0[0], a0[2], a0[3], a0[4], a0[1])

            attention_pair(0, 3, q0, k0, v0, a0[0], a0[2])
            v1 = proj_v(1, xt1)
            proj_qk_chunk(1, 1, xt1, q1, k1)
            norm_pair(0, 3, a0[0], a0[2], a0[3], a0[4], a0[1])
            a1 = alloc_attn(1)

            attention_pair(1, 0, q1, k1, v1, a1[0], a1[2])
            proj_qk_chunk(1, 2, xt1, q1, k1)
            out_proj_chunk(0, 0, a0[1])
            norm_pair(1, 0, a1[0], a1[2], a1[3], a1[4], a1[1])

            attention_pair(1, 1, q1, k1, v1, a1[0], a1[2])
            proj_qk_chunk(1, 3, xt1, q1, k1)
            out_proj_chunk(0, 1, a0[1])
            norm_pair(1, 1, a1[0], a1[2], a1[3], a1[4], a1[1])

            attention_pair(1, 2, q1, k1, v1, a1[0], a1[2])
            out_proj_chunk(0, 2, a0[1])
            norm_pair(1, 2, a1[0], a1[2], a1[3], a1[4], a1[1])

            attention_pair(1, 3, q1, k1, v1, a1[0], a1[2])
            out_proj_chunk(0, 3, a0[1])
            norm_pair(1, 3, a1[0], a1[2], a1[3], a1[4], a1[1])

            out_proj_tail(1, (0, 1), a1[1])
            out_proj_tail(1, (2, 3), a1[1])

    nc.compile()
    return nc


def _get_nc():
    if "nc" not in _CACHE:
        _CACHE["nc"] = _build_nc()
    return _CACHE["nc"]


def kernel(X, Wq, bq, Wk, bk, Wv, bv, Wo, bo):
    from concourse.bass_utils import run_bass_kernel_spmd

    nc = _get_nc()
    bf16 = ml_dtypes.bfloat16

    Xf = np.asarray(X, np.float32).reshape(B * T, N, D)
    XT_all = np.ascontiguousarray(Xf.transpose(0, 2, 1)).astype(bf16)  # [16, D, N]
    WQT = np.ascontiguousarray(np.asarray(Wq, np.float32).T * S_SCALE).astype(bf16)
    WKT = np.ascontiguousarray(np.asarray(Wk, np.float32).T).astype(bf16)
    WVT = np.ascontiguousarray(np.asarray(Wv, np.float32).T).astype(bf16)
    WOT = np.ascontiguousarray(np.asarray(Wo, np.float32).T).astype(bf16)
    bo_eff = (np.asarray(bo, np.float32)
              + np.asarray(Wo, np.float32) @ np.asarray(bv, np.float32))
    BQa = (np.asarray(bq, np.float32) * S_SCALE).reshape(NDT, P, 1)
    BKa = np.asarray(bk, np.float32).reshape(NDT, P, 1)
    BOa = bo_eff.reshape(NDT, P, 1)

    in_maps = []
    for c in range(NCORES):
        in_maps.append({
            "XT": np.ascontiguousarray(XT_all[c * NSLICE:(c + 1) * NSLICE]),
            "WQT": WQT, "WKT": WKT, "WVT": WVT, "WOT": WOT,
            "BQ": BQa, "BK": BKa, "BO": BOa,
        })

    trace = bool(int(os.environ.get("KERNEL_TRACE", "0")))
    kwargs = {}
    if trace:
        import tempfile
        kwargs = {"trace": True, "tmpdir": tempfile.mkdtemp(prefix="ker_trace_")}
    res = run_bass_kernel_spmd(nc, in_maps, core_ids=list(range(NCORES)), **kwargs)
    _CACHE["last_exec_ns"] = res.exec_time_ns

    out = np.empty((B * T, N, D), np.float32)
    for c in range(NCORES):
        ot = np.asarray(res.results[c]["OT"], np.float32)  # [NSLICE, D, N]
        out[c * NSLICE:(c + 1) * NSLICE] = ot.transpose(0, 2, 1)
    return out.reshape(B, T, N, D)


# revision 18
# speedup vs baseline: 2.3853x; 1.1643x over previous
"""Multi-head self-attention (AdaptiveTemporalContrastEnhancement) on 8 TRN2 cores.

Key facts baked in:
- The temporal-difference bias delta_c is added uniformly along the softmax
  axis, so softmax cancels it exactly -> it is skipped entirely.
- max |logit| ~ 1.9, so softmax runs without max-subtraction.
- V bias + output bias fold into one effective output bias:
      out = A@(XWv^T + bv) Wo^T + bo = A@(XWv^T)Wo^T + (Wo bv + bo).
- 1/sqrt(dh) is folded into WQT/BQ host-side.
- Data parallel over the 16 (b,t) slices: 2 slices per core, no collectives.
- All matmuls in bf16 (1 cyc/row on PE); accumulation fp32 in PSUM.

Device layout per slice (all "T" = dim-major, tokens along the free axis):
  XT  [d, n]   : 4 x [128, 1024] sbuf tiles (host pre-transposed)
  QT,KT [e, n] : computed as W^T.T @ XT  (4 x [128,1024])
  V_pad [n, .] : token-major, padded per head to a [128,128] stationary:
                 even head h: V cols 0-63, ones col 64, zeros 65-127
                 odd  head h: ones col 0, zeros 1-63,  V cols 64-127
                 so the PV matmul puts head h's Z^T at partitions 64*(h%2)..+63
                 and the softmax denominator at row 64 (even) / row 0 (odd).
  S^T [kv, q]  : head-PAIR packed: one [128, 1024] psum tile holds both heads'
                 S^T for one (kv, q-half); the two S matmuls use disjoint PE
                 row groups (partitions 0-63 / 64-127) and run concurrently.
  Z^T [d, q]   : per (head, q-half) [128, 512] psum accum over kv; evacuated
                 (with denominator row) to sbuf zun per head block.
  denominators : reshaped by DMA into dall8[128, 64] (head h = 16 partitions
                 x 64 cols) so ONE short-free-dim reciprocal per head PAIR is
                 cheap; broadcast back via a DRAM bounce; TT-mult per head.
  O^T [e, n]   : out-proj from normalized Z^T; host transposes back.

The schedule is software-pipelined at instruction level: projection and
out-projection chunks are emitted between attention head-pairs so the PE
fills the idle left by the ACT-paced exp stream, keeping the PE busy (and
its HAM clock-gate warm) while both slices' attention runs back-to-back.
"""

import os
import numpy as np
import ml_dtypes

B, T, N, D = 2, 8, 1024, 512
H, DH = 8, 64
P = 128
NDT = D // P          # 4 d-tiles
NKV = N // P          # 8 kv tiles
NQH = N // 512        # 2 q halves
NCORES = 8
NSLICE = (B * T) // NCORES   # 2 slices per core
S_SCALE = float(1.0 / np.sqrt(DH))  # 0.125

_CACHE = {}


def _build_nc():
    import concourse.mybir as mybir
    from concourse import bacc
    from concourse.tile import TileContext
    import concourse.bass as bass

    f32, bf16 = mybir.dt.float32, mybir.dt.bfloat16
    nc = bacc.Bacc("TRN2", target_bir_lowering=False, debug=False)

    XT = nc.dram_tensor("XT", [NSLICE, D, N], bf16, kind="ExternalInput")
    WQT = nc.dram_tensor("WQT", [D, D], bf16, kind="ExternalInput")
    WKT = nc.dram_tensor("WKT", [D, D], bf16, kind="ExternalInput")
    WVT = nc.dram_tensor("WVT", [D, D], bf16, kind="ExternalInput")
    WOT = nc.dram_tensor("WOT", [D, D], bf16, kind="ExternalInput")
    BQ = nc.dram_tensor("BQ", [NDT, P, 1], f32, kind="ExternalInput")
    BK = nc.dram_tensor("BK", [NDT, P, 1], f32, kind="ExternalInput")
    BO = nc.dram_tensor("BO", [NDT, P, 1], f32, kind="ExternalInput")
    OT = nc.dram_tensor("OT", [NSLICE, D, N], f32, kind="ExternalOutput")

    Exp = mybir.ActivationFunctionType.Exp
    Mult = mybir.AluOpType.mult

    with TileContext(nc) as tc:
        with (
            tc.tile_pool(name="wpool", bufs=1) as wpool,
            tc.tile_pool(name="xpool", bufs=2) as xpool,
            tc.tile_pool(name="qkpool", bufs=2) as qkpool,
            tc.tile_pool(name="vpool", bufs=2) as vpool,
            tc.tile_pool(name="apool", bufs=4) as apool,
            tc.tile_pool(name="zpool", bufs=2) as zpool,
            tc.tile_pool(name="rpool", bufs=2) as rpool,
            tc.tile_pool(name="opool", bufs=3) as opool,
            tc.tile_pool(name="drpool", bufs=2, space="DRAM") as drpool,
            tc.tile_pool(name="ps_s", bufs=2, space="PSUM") as ps_s,
            tc.tile_pool(name="ps_z", bufs=2, space="PSUM") as ps_z,
            tc.tile_pool(name="ps_c", bufs=1, space="PSUM") as ps_c,
        ):
            # ---- persistent weights / biases (DMAs split for queue parallelism;
            #      wq/wk/bq/bk first so the first projection chunk starts early) ----
            w_sb, b_sb = {}, {}

            def emit_w(name, dram):
                # one 3D-AP DMA per weight: [512,512] dram -> [128, 4*512] sbuf
                t = wpool.tile([P, NDT * 512], bf16, tag=name, name=f"w_{name}")
                w_sb[name] = t
                nc.sync.dma_start(
                    out=t[:, :].rearrange("p (dt e) -> p dt e", e=512),
                    in_=dram[:, :].rearrange("(dt p) e -> p dt e", p=P),
                )

            def emit_b(name, dram):
                t = wpool.tile([P, NDT], f32, tag=name, name=f"b_{name}")
                b_sb[name] = t
                nc.sync.dma_start(
                    out=t[:, :],
                    in_=dram[:, :, :].rearrange("et p one -> p (et one)"),
                )

            def load_x(s):
                # one 1MB DMA: large transfers reach full fabric bandwidth
                xt = xpool.tile([P, NDT * N], bf16, tag="xt", name=f"xt_{s}")
                nc.sync.dma_start(
                    out=xt[:, :].rearrange("p (dt n) -> p dt n", n=N),
                    in_=XT[s].rearrange("(dt p) n -> p dt n", p=P),
                )
                return xt

            def gen_qk_chunk(s, et, xt, qt, kt):
                """Filler generator: yields after each matmul so attention
                can weave these into the exp-paced stream one MM at a time."""
                for dst, wname, bname in ((qt, "wq", "bq"), (kt, "wk", "bk")):
                    w = w_sb[wname]
                    ps = ps_c.tile([P, N], f32, tag="c", name=f"psc_{wname}_{s}_{et}")
                    for dt_ in range(NDT):
                        for qh in range(NQH):
                            nc.tensor.matmul(
                                ps[:, qh * 512:(qh + 1) * 512],
                                lhsT=w[:, dt_ * 512 + et * P: dt_ * 512 + (et + 1) * P],
                                rhs=xt[:, dt_ * N + qh * 512: dt_ * N + qh * 512 + 512],
                                start=(dt_ == 0), stop=(dt_ == NDT - 1),
                            )
                            if dt_ == NDT - 1:
                                # evac inside the same pop as the last matmul so
                                # consumers emitted next step see it ordered
                                nc.vector.tensor_scalar_add(
                                    dst[:, et * N + qh * 512: et * N + qh * 512 + 512],
                                    ps[:, qh * 512:(qh + 1) * 512],
                                    b_sb[bname][:, et:et + 1],
                                )
                            yield

            def gen_op_chunk(s, et, zt):
                ps = ps_c.tile([P, N], f32, tag="c", name=f"psc_o_{s}_{et}")
                o_sb = opool.tile([P, N], f32, tag="o", name=f"o_{s}_{et}")
                for dd in range(NDT):
                    for qh in range(NQH):
                        nc.tensor.matmul(
                            ps[:, qh * 512:(qh + 1) * 512],
                            lhsT=w_sb["wo"][:, dd * 512 + et * P: dd * 512 + (et + 1) * P],
                            rhs=zt[:, dd * N + qh * 512: dd * N + qh * 512 + 512],
                            start=(dd == 0), stop=(dd == NDT - 1),
                        )
                        if dd == NDT - 1:
                            nc.vector.tensor_scalar_add(
                                o_sb[:, qh * 512:(qh + 1) * 512],
                                ps[:, qh * 512:(qh + 1) * 512], b_sb["bo"][:, et:et + 1])
                            if qh == NQH - 1:
                                nc.sync.dma_start(
                                    out=OT[s, et * P:(et + 1) * P, :], in_=o_sb)
                        yield

            def proj_qk_chunk(s, et, xt, qt, kt):
                for dst, wname, bname in ((qt, "wq", "bq"), (kt, "wk", "bk")):
                    w = w_sb[wname]
                    ps = ps_s.tile([P, N], f32, tag="s", name=f"ps_{wname}_{s}_{et}")
                    for dt_ in range(NDT):
                        for qh in range(NQH):
                            nc.tensor.matmul(
                                ps[:, qh * 512:(qh + 1) * 512],
                                lhsT=w[:, dt_ * 512 + et * P: dt_ * 512 + (et + 1) * P],
                                rhs=xt[:, dt_ * N + qh * 512: dt_ * N + qh * 512 + 512],
                                start=(dt_ == 0), stop=(dt_ == NDT - 1),
                            )
                    for qh in range(NQH):
                        nc.vector.tensor_scalar_add(
                            dst[:, et * N + qh * 512: et * N + qh * 512 + 512],
                            ps[:, qh * 512:(qh + 1) * 512],
                            b_sb[bname][:, et:et + 1],
                        )

            def proj_v(s, xt):
                v_sb = vpool.tile([P, NKV * H * P], bf16, tag="v", name=f"v_{s}")
                vz = v_sb.rearrange("p (b r) -> p b r", r=256)
                nc.gpsimd.memset(vz[:, :, 65:128], 0.0)    # even-head pad
                nc.gpsimd.memset(vz[:, :, 129:192], 0.0)   # odd-head pad
                nc.vector.memset(vz[:, :, 64:65], 1.0)     # even-head ones col
                nc.vector.memset(vz[:, :, 128:129], 1.0)   # odd-head ones col
                for kv in range(NKV):
                    ps = ps_s.tile([P, N], f32, tag="s", name=f"ps_v_{s}_{kv}")
                    for dt_ in range(NDT):
                        nc.tensor.matmul(
                            ps[:, 0:512],
                            lhsT=xt[:, dt_ * N + kv * P: dt_ * N + (kv + 1) * P],
                            rhs=w_sb["wv"][:, dt_ * 512:(dt_ + 1) * 512],
                            start=(dt_ == 0), stop=(dt_ == NDT - 1),
                        )
                    vblk = v_sb[:, kv * 1024:(kv + 1) * 1024].rearrange(
                        "p (hp r) -> p hp r", r=256)
                    psh = ps[:, 0:512].rearrange("p (hp c) -> p hp c", c=128)
                    nc.vector.tensor_copy(vblk[:, :, 0:64], psh[:, :, 0:64])
                    nc.vector.tensor_copy(vblk[:, :, 192:256], psh[:, :, 64:128])
                return v_sb

            def attention_pair(s, j, qt, kt, v_sb, zun, dall8, filler=None):
                """Heads 2j, 2j+1: S matmuls packed into disjoint PE row
                groups; one exp covers both heads; PV per head/q-half. One
                filler matmul is woven in after each kv step."""
                et = j
                for qh in range(NQH):
                    zs = [ps_z.tile([P, 512], f32, tag="z", name=f"z_{s}_{j}_{qh}_{p_}")
                          for p_ in range(2)]
                    for kv in range(NKV):
                        s_ps = ps_s.tile([P, N], f32, tag="s", name=f"s_{s}_{j}_{qh}_{kv}")
                        for p_ in range(2):
                            pb = 64 * p_
                            nc.tensor.matmul(
                                s_ps[:, p_ * 512:(p_ + 1) * 512],
                                lhsT=kt[pb:pb + 64, et * N + kv * P: et * N + (kv + 1) * P],
                                rhs=qt[pb:pb + 64, et * N + qh * 512: et * N + qh * 512 + 512],
                                start=True, stop=True,
                            )
                        at = apool.tile([P, N], bf16, tag="at", name=f"at_{s}_{j}_{qh}_{kv}")
                        nc.scalar.activation(at, s_ps, Exp)
                        for p_ in range(2):
                            h = 2 * j + p_
                            nc.tensor.matmul(
                                zs[p_],
                                lhsT=v_sb[:, kv * 1024 + h * P: kv * 1024 + (h + 1) * P],
                                rhs=at[:, p_ * 512:(p_ + 1) * 512],
                                start=(kv == 0), stop=(kv == NKV - 1),
                            )
                        if filler is not None:
                            next(filler, None)
                    for p_ in range(2):
                        h = 2 * j + p_
                        nc.vector.tensor_copy(
                            zun[:, h * N + qh * 512: h * N + qh * 512 + 512], zs[p_])
                for p_ in range(2):
                    h = 2 * j + p_
                    dr = 64 if h % 2 == 0 else 0
                    # reshape-gather the denom row into dall8[16h:16h+16, 0:64]
                    nc.gpsimd.dma_start(  # gpsimd: casting DMA bf16 -> f32
                        out=dall8[16 * h:16 * (h + 1), :],
                        in_=zun[dr:dr + 1, h * N:(h + 1) * N])

            def norm_pair(s, j, zun, dall8, rall8, rdram, zt):
                # short-free-dim reciprocal over the pair's 32 partitions
                nc.vector.reciprocal(rall8[32 * j:32 * (j + 1), :],
                                     dall8[32 * j:32 * (j + 1), :])
                nc.sync.dma_start(out=rdram[32 * j:32 * (j + 1), :],
                                  in_=rall8[32 * j:32 * (j + 1), :])
                rbc = rpool.tile([P, N], f32, tag="rbc", name=f"rbc_{s}_{j}")
                for p_ in range(2):
                    h = 2 * j + p_
                    base = rdram[0:1, 0:1]
                    nc.sync.dma_start(
                        out=rbc[64 * p_:64 * p_ + 64, :],
                        in_=bass.AP(tensor=base.tensor, offset=base.offset + h * N,
                                    ap=[[0, 64], [1, N]]),
                    )
                for p_ in range(2):
                    h = 2 * j + p_
                    pb = 64 * p_
                    nc.vector.tensor_tensor(
                        out=zt[pb:pb + 64, j * N:(j + 1) * N],
                        in0=zun[pb:pb + 64, h * N:(h + 1) * N],
                        in1=rbc[pb:pb + 64, :], op=Mult,
                    )

            def out_proj_tail(s, ets, zt):
                """Out-proj for two e-tiles with the dd=3 (last head pair)
                contraction deferred, so these matmuls start before the last
                norm_pair's TT-mults have produced zt block 3."""
                pss = {}
                for et in ets:
                    ps = ps_s.tile([P, N], f32, tag="s", name=f"ps_ot_{s}_{et}")
                    pss[et] = ps
                    for dd in range(NDT - 1):
                        for qh in range(NQH):
                            nc.tensor.matmul(
                                ps[:, qh * 512:(qh + 1) * 512],
                                lhsT=w_sb["wo"][:, dd * 512 + et * P: dd * 512 + (et + 1) * P],
                                rhs=zt[:, dd * N + qh * 512: dd * N + qh * 512 + 512],
                                start=(dd == 0), stop=False,
                            )
                for et in ets:
                    ps = pss[et]
                    dd = NDT - 1
                    for qh in range(NQH):
                        nc.tensor.matmul(
                            ps[:, qh * 512:(qh + 1) * 512],
                            lhsT=w_sb["wo"][:, dd * 512 + et * P: dd * 512 + (et + 1) * P],
                            rhs=zt[:, dd * N + qh * 512: dd * N + qh * 512 + 512],
                            start=False, stop=True,
                        )
                    o_sb = opool.tile([P, N], f32, tag="o", name=f"o_{s}_{et}")
                    for qh in range(NQH):
                        nc.vector.tensor_scalar_add(
                            o_sb[:, qh * 512:(qh + 1) * 512],
                            ps[:, qh * 512:(qh + 1) * 512], b_sb["bo"][:, et:et + 1])
                    nc.sync.dma_start(out=OT[s, et * P:(et + 1) * P, :], in_=o_sb)

            def out_proj_chunk(s, et, zt):
                ps = ps_s.tile([P, N], f32, tag="s", name=f"ps_o_{s}_{et}")
                for dd in range(NDT):
                    for qh in range(NQH):
                        nc.tensor.matmul(
                            ps[:, qh * 512:(qh + 1) * 512],
                            lhsT=w_sb["wo"][:, dd * 512 + et * P: dd * 512 + (et + 1) * P],
                            rhs=zt[:, dd * N + qh * 512: dd * N + qh * 512 + 512],
                            start=(dd == 0), stop=(dd == NDT - 1),
                        )
                o_sb = opool.tile([P, N], f32, tag="o", name=f"o_{s}_{et}")
                for qh in range(NQH):
                    nc.vector.tensor_scalar_add(
                        o_sb[:, qh * 512:(qh + 1) * 512],
                        ps[:, qh * 512:(qh + 1) * 512], b_sb["bo"][:, et:et + 1])
                nc.sync.dma_start(out=OT[s, et * P:(et + 1) * P, :], in_=o_sb)

            def alloc_attn(s):
                zun = zpool.tile([P, H * N], bf16, tag="zun", name=f"zun_{s}")
                zt = zpool.tile([P, NDT * N], bf16, tag="zt", name=f"zt_{s}")
                dall8 = rpool.tile([P, 64], f32, tag="dall", name=f"dall_{s}")
                rall8 = rpool.tile([P, 64], f32, tag="rall", name=f"rall_{s}")
                rdram = drpool.tile([P, 64], f32, tag="rdram", name=f"rdram_{s}")
                return zun, zt, dall8, rall8, rdram

            # ---- schedule ----
            from itertools import chain

            emit_w("wq", WQT)
            emit_w("wk", WKT)
            emit_b("bq", BQ)
            emit_b("bk", BK)
            xt0 = load_x(0)
            emit_w("wv", WVT)
            emit_w("wo", WOT)
            emit_b("bo", BO)
            xt1 = load_x(1)

            q0 = qkpool.tile([P, NDT * N], bf16, tag="qt", name="qt_0")
            k0 = qkpool.tile([P, NDT * N], bf16, tag="kt", name="kt_0")
            q1 = qkpool.tile([P, NDT * N], bf16, tag="qt", name="qt_1")
            k1 = qkpool.tile([P, NDT * N], bf16, tag="kt", name="kt_1")

            # startup (ACT idle): first QK chunk + BOTH slices' V projections
            proj_qk_chunk(0, 0, xt0, q0, k0)
            v0 = proj_v(0, xt0)
            v1 = proj_v(1, xt1)
            a0 = alloc_attn(0)
            a1 = alloc_attn(1)

            # filler chain: exactly 8 pairs x 16 kv-steps = 128 matmuls
            F = chain(
                gen_qk_chunk(0, 1, xt0, q0, k0),
                gen_qk_chunk(0, 2, xt0, q0, k0),
                gen_qk_chunk(0, 3, xt0, q0, k0),
                gen_qk_chunk(1, 0, xt1, q1, k1),
                gen_qk_chunk(1, 1, xt1, q1, k1),
                gen_qk_chunk(1, 2, xt1, q1, k1),
                gen_qk_chunk(1, 3, xt1, q1, k1),
                gen_op_chunk(0, 0, a0[1]),
                gen_op_chunk(0, 1, a0[1]),
            )

            for j in range(NDT):
                attention_pair(0, j, q0, k0, v0, a0[0], a0[2], filler=F)
                norm_pair(0, j, a0[0], a0[2], a0[3], a0[4], a0[1])
            for j in range(NDT):
                attention_pair(1, j, q1, k1, v1, a1[0], a1[2], filler=F)
                norm_pair(1, j, a1[0], a1[2], a1[3], a1[4], a1[1])

            for _ in F:  # drain any leftover fillers
                pass
            for _ in gen_op_chunk(0, 2, a0[1]):
                pass
            for _ in gen_op_chunk(0, 3, a0[1]):
                pass
            out_proj_tail(1, (0, 1), a1[1])
            out_proj_tail(1, (2, 3), a1[1])

    nc.compile()
    return nc


def _get_nc():
    if "nc" not in _CACHE:
        _CACHE["nc"] = _build_nc()
    return _CACHE["nc"]


def kernel(X, Wq, bq, Wk, bk, Wv, bv, Wo, bo):
    from concourse.bass_utils import run_bass_kernel_spmd

    nc = _get_nc()
    bf16 = ml_dtypes.bfloat16

    Xf = np.asarray(X, np.float32).reshape(B * T, N, D)
    XT_all = np.ascontiguousarray(Xf.transpose(0, 2, 1)).astype(bf16)  # [16, D, N]
    WQT = np.ascontiguousarray(np.asarray(Wq, np.float32).T * S_SCALE).astype(bf16)
    WKT = np.ascontiguousarray(np.asarray(Wk, np.float32).T).astype(bf16)
    WVT = np.ascontiguousarray(np.asarray(Wv, np.float32).T).astype(bf16)
    WOT = np.ascontiguousarray(np.asarray(Wo, np.float32).T).astype(bf16)
    bo_eff = (np.asarray(bo, np.float32)
              + np.asarray(Wo, np.float32) @ np.asarray(bv, np.float32))
    BQa = (np.asarray(bq, np.float32) * S_SCALE).reshape(NDT, P, 1)
    BKa = np.asarray(bk, np.float32).reshape(NDT, P, 1)
    BOa = bo_eff.reshape(NDT, P, 1)

    in_maps = []
    for c in range(NCORES):
        in_maps.append({
            "XT": np.ascontiguousarray(XT_all[c * NSLICE:(c + 1) * NSLICE]),
            "WQT": WQT, "WKT": WKT, "WVT": WVT, "WOT": WOT,
            "BQ": BQa, "BK": BKa, "BO": BOa,
        })

    trace = bool(int(os.environ.get("KERNEL_TRACE", "0")))
    kwargs = {}
    if trace:
        import tempfile
        kwargs = {"trace": True, "tmpdir": tempfile.mkdtemp(prefix="ker_trace_")}
    res = run_bass_kernel_spmd(nc, in_maps, core_ids=list(range(NCORES)), **kwargs)
    _CACHE["last_exec_ns"] = res.exec_time_ns

    out = np.empty((B * T, N, D), np.float32)
    for c in range(NCORES):
        ot = np.asarray(res.results[c]["OT"], np.float32)  # [NSLICE, D, N]
        out[c * NSLICE:(c + 1) * NSLICE] = ot.transpose(0, 2, 1)
    return out.reshape(B, T, N, D)


# revision 19
# speedup vs baseline: 2.3994x; 1.0059x over previous
"""Multi-head self-attention (AdaptiveTemporalContrastEnhancement) on 8 TRN2 cores.

Key facts baked in:
- The temporal-difference bias delta_c is added uniformly along the softmax
  axis, so softmax cancels it exactly -> it is skipped entirely.
- max |logit| ~ 1.9, so softmax runs without max-subtraction.
- V bias + output bias fold into one effective output bias:
      out = A@(XWv^T + bv) Wo^T + bo = A@(XWv^T)Wo^T + (Wo bv + bo).
- 1/sqrt(dh) is folded into WQT/BQ host-side.
- Data parallel over the 16 (b,t) slices: 2 slices per core, no collectives.
- All matmuls in bf16 (1 cyc/row on PE); accumulation fp32 in PSUM.

Device layout per slice (all "T" = dim-major, tokens along the free axis):
  XT  [d, n]   : 4 x [128, 1024] sbuf tiles (host pre-transposed)
  QT,KT [e, n] : computed as W^T.T @ XT  (4 x [128,1024])
  V_pad [n, .] : token-major, padded per head to a [128,128] stationary:
                 even head h: V cols 0-63, ones col 64, zeros 65-127
                 odd  head h: ones col 0, zeros 1-63,  V cols 64-127
                 so the PV matmul puts head h's Z^T at partitions 64*(h%2)..+63
                 and the softmax denominator at row 64 (even) / row 0 (odd).
  S^T [kv, q]  : head-PAIR packed: one [128, 1024] psum tile holds both heads'
                 S^T for one (kv, q-half); the two S matmuls use disjoint PE
                 row groups (partitions 0-63 / 64-127) and run concurrently.
  Z^T [d, q]   : per (head, q-half) [128, 512] psum accum over kv; evacuated
                 (with denominator row) to sbuf zun per head block.
  denominators : reshaped by DMA into dall8[128, 64] (head h = 16 partitions
                 x 64 cols) so ONE short-free-dim reciprocal per head PAIR is
                 cheap; broadcast back via a DRAM bounce; TT-mult per head.
  O^T [e, n]   : out-proj from normalized Z^T; host transposes back.

The schedule is software-pipelined at instruction level: projection and
out-projection chunks are emitted between attention head-pairs so the PE
fills the idle left by the ACT-paced exp stream, keeping the PE busy (and
its HAM clock-gate warm) while both slices' attention runs back-to-back.
"""

import os
import numpy as np
import ml_dtypes

B, T, N, D = 2, 8, 1024, 512
H, DH = 8, 64
P = 128
NDT = D // P          # 4 d-tiles
NKV = N // P          # 8 kv tiles
NQH = N // 512        # 2 q halves
NCORES = 8
NSLICE = (B * T) // NCORES   # 2 slices per core
S_SCALE = float(1.0 / np.sqrt(DH))  # 0.125

_CACHE = {}


def _build_nc():
    import concourse.mybir as mybir
    from concourse import bacc
    from concourse.tile import TileContext
    import concourse.bass as bass

    f32, bf16 = mybir.dt.float32, mybir.dt.bfloat16
    nc = bacc.Bacc("TRN2", target_bir_lowering=False, debug=False)

    XT = nc.dram_tensor("XT", [NSLICE, D, N], bf16, kind="ExternalInput")
    WQT = nc.dram_tensor("WQT", [D, D], bf16, kind="ExternalInput")
    WKT = nc.dram_tensor("WKT", [D, D], bf16, kind="ExternalInput")
    WVT = nc.dram_tensor("WVT", [D, D], bf16, kind="ExternalInput")
    WOT = nc.dram_tensor("WOT", [D, D], bf16, kind="ExternalInput")
    BQ = nc.dram_tensor("BQ", [NDT, P, 1], f32, kind="ExternalInput")
    BK = nc.dram_tensor("BK", [NDT, P, 1], f32, kind="ExternalInput")
    BO = nc.dram_tensor("BO", [NDT, P, 1], f32, kind="ExternalInput")
    OT = nc.dram_tensor("OT", [NSLICE, D, N], f32, kind="ExternalOutput")

    Exp = mybir.ActivationFunctionType.Exp
    Mult = mybir.AluOpType.mult

    with TileContext(nc) as tc:
        with (
            tc.tile_pool(name="wpool", bufs=1) as wpool,
            tc.tile_pool(name="xpool", bufs=2) as xpool,
            tc.tile_pool(name="qkpool", bufs=2) as qkpool,
            tc.tile_pool(name="vpool", bufs=2) as vpool,
            tc.tile_pool(name="apool", bufs=4) as apool,
            tc.tile_pool(name="zpool", bufs=2) as zpool,
            tc.tile_pool(name="rpool", bufs=2) as rpool,
            tc.tile_pool(name="opool", bufs=3) as opool,
            tc.tile_pool(name="drpool", bufs=2, space="DRAM") as drpool,
            tc.tile_pool(name="ps_s", bufs=2, space="PSUM") as ps_s,
            tc.tile_pool(name="ps_z", bufs=2, space="PSUM") as ps_z,
            tc.tile_pool(name="ps_c", bufs=1, space="PSUM") as ps_c,
        ):
            # ---- persistent weights / biases (DMAs split for queue parallelism;
            #      wq/wk/bq/bk first so the first projection chunk starts early) ----
            w_sb, b_sb = {}, {}

            def emit_w(name, dram):
                # one 3D-AP DMA per weight: [512,512] dram -> [128, 4*512] sbuf
                t = wpool.tile([P, NDT * 512], bf16, tag=name, name=f"w_{name}")
                w_sb[name] = t
                nc.sync.dma_start(
                    out=t[:, :].rearrange("p (dt e) -> p dt e", e=512),
                    in_=dram[:, :].rearrange("(dt p) e -> p dt e", p=P),
                )

            def emit_b(name, dram):
                t = wpool.tile([P, NDT], f32, tag=name, name=f"b_{name}")
                b_sb[name] = t
                nc.sync.dma_start(
                    out=t[:, :],
                    in_=dram[:, :, :].rearrange("et p one -> p (et one)"),
                )

            def load_x(s):
                # one 1MB DMA: large transfers reach full fabric bandwidth
                xt = xpool.tile([P, NDT * N], bf16, tag="xt", name=f"xt_{s}")
                nc.sync.dma_start(
                    out=xt[:, :].rearrange("p (dt n) -> p dt n", n=N),
                    in_=XT[s].rearrange("(dt p) n -> p dt n", p=P),
                )
                return xt

            def gen_qk_chunk(s, et, xt, qt, kt):
                """Filler generator: yields after each matmul so attention
                can weave these into the exp-paced stream one MM at a time."""
                for dst, wname, bname in ((qt, "wq", "bq"), (kt, "wk", "bk")):
                    w = w_sb[wname]
                    ps = ps_c.tile([P, N], f32, tag="c", name=f"psc_{wname}_{s}_{et}")
                    for dt_ in range(NDT):
                        for qh in range(NQH):
                            nc.tensor.matmul(
                                ps[:, qh * 512:(qh + 1) * 512],
                                lhsT=w[:, dt_ * 512 + et * P: dt_ * 512 + (et + 1) * P],
                                rhs=xt[:, dt_ * N + qh * 512: dt_ * N + qh * 512 + 512],
                                start=(dt_ == 0), stop=(dt_ == NDT - 1),
                            )
                            if dt_ == NDT - 1:
                                # evac inside the same pop as the last matmul so
                                # consumers emitted next step see it ordered
                                nc.vector.tensor_scalar_add(
                                    dst[:, et * N + qh * 512: et * N + qh * 512 + 512],
                                    ps[:, qh * 512:(qh + 1) * 512],
                                    b_sb[bname][:, et:et + 1],
                                )
                            yield

            def gen_op_chunk(s, et, zt):
                ps = ps_c.tile([P, N], f32, tag="c", name=f"psc_o_{s}_{et}")
                o_sb = opool.tile([P, N], f32, tag="o", name=f"o_{s}_{et}")
                for dd in range(NDT):
                    for qh in range(NQH):
                        nc.tensor.matmul(
                            ps[:, qh * 512:(qh + 1) * 512],
                            lhsT=w_sb["wo"][:, dd * 512 + et * P: dd * 512 + (et + 1) * P],
                            rhs=zt[:, dd * N + qh * 512: dd * N + qh * 512 + 512],
                            start=(dd == 0), stop=(dd == NDT - 1),
                        )
                        if dd == NDT - 1:
                            nc.vector.tensor_scalar_add(
                                o_sb[:, qh * 512:(qh + 1) * 512],
                                ps[:, qh * 512:(qh + 1) * 512], b_sb["bo"][:, et:et + 1])
                            if qh == NQH - 1:
                                nc.sync.dma_start(
                                    out=OT[s, et * P:(et + 1) * P, :], in_=o_sb)
                        yield

            def proj_qk_chunk(s, et, xt, qt, kt):
                for dst, wname, bname in ((qt, "wq", "bq"), (kt, "wk", "bk")):
                    w = w_sb[wname]
                    ps = ps_s.tile([P, N], f32, tag="s", name=f"ps_{wname}_{s}_{et}")
                    for dt_ in range(NDT):
                        for qh in range(NQH):
                            nc.tensor.matmul(
                                ps[:, qh * 512:(qh + 1) * 512],
                                lhsT=w[:, dt_ * 512 + et * P: dt_ * 512 + (et + 1) * P],
                                rhs=xt[:, dt_ * N + qh * 512: dt_ * N + qh * 512 + 512],
                                start=(dt_ == 0), stop=(dt_ == NDT - 1),
                            )
                    for qh in range(NQH):
                        nc.vector.tensor_scalar_add(
                            dst[:, et * N + qh * 512: et * N + qh * 512 + 512],
                            ps[:, qh * 512:(qh + 1) * 512],
                            b_sb[bname][:, et:et + 1],
                        )

            def proj_v(s, xt):
                v_sb = vpool.tile([P, NKV * H * P], bf16, tag="v", name=f"v_{s}")
                vz = v_sb.rearrange("p (b r) -> p b r", r=256)
                nc.gpsimd.memset(vz[:, :, 65:128], 0.0)    # even-head pad
                nc.gpsimd.memset(vz[:, :, 129:192], 0.0)   # odd-head pad
                nc.vector.memset(vz[:, :, 64:65], 1.0)     # even-head ones col
                nc.vector.memset(vz[:, :, 128:129], 1.0)   # odd-head ones col
                for kv in range(NKV):
                    ps = ps_s.tile([P, N], f32, tag="s", name=f"ps_v_{s}_{kv}")
                    for dt_ in range(NDT):
                        nc.tensor.matmul(
                            ps[:, 0:512],
                            lhsT=xt[:, dt_ * N + kv * P: dt_ * N + (kv + 1) * P],
                            rhs=w_sb["wv"][:, dt_ * 512:(dt_ + 1) * 512],
                            start=(dt_ == 0), stop=(dt_ == NDT - 1),
                        )
                    vblk = v_sb[:, kv * 1024:(kv + 1) * 1024].rearrange(
                        "p (hp r) -> p hp r", r=256)
                    psh = ps[:, 0:512].rearrange("p (hp c) -> p hp c", c=128)
                    nc.vector.tensor_copy(vblk[:, :, 0:64], psh[:, :, 0:64])
                    nc.vector.tensor_copy(vblk[:, :, 192:256], psh[:, :, 64:128])
                return v_sb

            def attention_pair(s, j, qt, kt, v_sb, zun, dall8, filler=None):
                """Heads 2j, 2j+1: S matmuls packed into disjoint PE row
                groups; one exp covers both heads; PV per head/q-half. One
                filler matmul is woven in after each kv step."""
                et = j
                for qh in range(NQH):
                    zs = [ps_z.tile([P, 512], f32, tag="z", name=f"z_{s}_{j}_{qh}_{p_}")
                          for p_ in range(2)]
                    for kv in range(NKV):
                        s_ps = ps_s.tile([P, N], f32, tag="s", name=f"s_{s}_{j}_{qh}_{kv}")
                        for p_ in range(2):
                            pb = 64 * p_
                            nc.tensor.matmul(
                                s_ps[:, p_ * 512:(p_ + 1) * 512],
                                lhsT=kt[pb:pb + 64, et * N + kv * P: et * N + (kv + 1) * P],
                                rhs=qt[pb:pb + 64, et * N + qh * 512: et * N + qh * 512 + 512],
                                start=True, stop=True,
                            )
                        at = apool.tile([P, N], bf16, tag="at", name=f"at_{s}_{j}_{qh}_{kv}")
                        nc.scalar.activation(at, s_ps, Exp)
                        for p_ in range(2):
                            h = 2 * j + p_
                            nc.tensor.matmul(
                                zs[p_],
                                lhsT=v_sb[:, kv * 1024 + h * P: kv * 1024 + (h + 1) * P],
                                rhs=at[:, p_ * 512:(p_ + 1) * 512],
                                start=(kv == 0), stop=(kv == NKV - 1),
                            )
                        if filler is not None:
                            next(filler, None)
                    for p_ in range(2):
                        h = 2 * j + p_
                        nc.vector.tensor_copy(
                            zun[:, h * N + qh * 512: h * N + qh * 512 + 512], zs[p_])
                for p_ in range(2):
                    h = 2 * j + p_
                    dr = 64 if h % 2 == 0 else 0
                    # reshape-gather the denom row into dall8[16h:16h+16, 0:64]
                    nc.gpsimd.dma_start(  # gpsimd: casting DMA bf16 -> f32
                        out=dall8[16 * h:16 * (h + 1), :],
                        in_=zun[dr:dr + 1, h * N:(h + 1) * N])

            def norm_pair(s, j, zun, dall8, rall8, rdram, zt):
                # short-free-dim reciprocal over the pair's 32 partitions
                nc.vector.reciprocal(rall8[32 * j:32 * (j + 1), :],
                                     dall8[32 * j:32 * (j + 1), :])
                nc.sync.dma_start(out=rdram[32 * j:32 * (j + 1), :],
                                  in_=rall8[32 * j:32 * (j + 1), :])
                rbc = rpool.tile([P, N], f32, tag="rbc", name=f"rbc_{s}_{j}")
                for p_ in range(2):
                    h = 2 * j + p_
                    base = rdram[0:1, 0:1]
                    nc.sync.dma_start(
                        out=rbc[64 * p_:64 * p_ + 64, :],
                        in_=bass.AP(tensor=base.tensor, offset=base.offset + h * N,
                                    ap=[[0, 64], [1, N]]),
                    )
                for p_ in range(2):
                    h = 2 * j + p_
                    pb = 64 * p_
                    nc.vector.tensor_tensor(
                        out=zt[pb:pb + 64, j * N:(j + 1) * N],
                        in0=zun[pb:pb + 64, h * N:(h + 1) * N],
                        in1=rbc[pb:pb + 64, :], op=Mult,
                    )

            def out_proj_tail(s, ets, zt):
                """Out-proj for two e-tiles with the dd=3 (last head pair)
                contraction deferred, so these matmuls start before the last
                norm_pair's TT-mults have produced zt block 3."""
                pss = {}
                for et in ets:
                    ps = ps_s.tile([P, N], f32, tag="s", name=f"ps_ot_{s}_{et}")
                    pss[et] = ps
                    for dd in range(NDT - 1):
                        for qh in range(NQH):
                            nc.tensor.matmul(
                                ps[:, qh * 512:(qh + 1) * 512],
                                lhsT=w_sb["wo"][:, dd * 512 + et * P: dd * 512 + (et + 1) * P],
                                rhs=zt[:, dd * N + qh * 512: dd * N + qh * 512 + 512],
                                start=(dd == 0), stop=False,
                            )
                for et in ets:
                    ps = pss[et]
                    dd = NDT - 1
                    for qh in range(NQH):
                        nc.tensor.matmul(
                            ps[:, qh * 512:(qh + 1) * 512],
                            lhsT=w_sb["wo"][:, dd * 512 + et * P: dd * 512 + (et + 1) * P],
                            rhs=zt[:, dd * N + qh * 512: dd * N + qh * 512 + 512],
                            start=False, stop=True,
                        )
                    o_sb = opool.tile([P, N], f32, tag="o", name=f"o_{s}_{et}")
                    for qh in range(NQH):
                        nc.vector.tensor_scalar_add(
                            o_sb[:, qh * 512:(qh + 1) * 512],
                            ps[:, qh * 512:(qh + 1) * 512], b_sb["bo"][:, et:et + 1])
                    nc.sync.dma_start(out=OT[s, et * P:(et + 1) * P, :], in_=o_sb)

            def out_proj_chunk(s, et, zt):
                ps = ps_s.tile([P, N], f32, tag="s", name=f"ps_o_{s}_{et}")
                for dd in range(NDT):
                    for qh in range(NQH):
                        nc.tensor.matmul(
                            ps[:, qh * 512:(qh + 1) * 512],
                            lhsT=w_sb["wo"][:, dd * 512 + et * P: dd * 512 + (et + 1) * P],
                            rhs=zt[:, dd * N + qh * 512: dd * N + qh * 512 + 512],
                            start=(dd == 0), stop=(dd == NDT - 1),
                        )
                o_sb = opool.tile([P, N], f32, tag="o", name=f"o_{s}_{et}")
                for qh in range(NQH):
                    nc.vector.tensor_scalar_add(
                        o_sb[:, qh * 512:(qh + 1) * 512],
                        ps[:, qh * 512:(qh + 1) * 512], b_sb["bo"][:, et:et + 1])
                nc.sync.dma_start(out=OT[s, et * P:(et + 1) * P, :], in_=o_sb)

            def alloc_attn(s):
                zun = zpool.tile([P, H * N], bf16, tag="zun", name=f"zun_{s}")
                zt = zpool.tile([P, NDT * N], bf16, tag="zt", name=f"zt_{s}")
                dall8 = rpool.tile([P, 64], f32, tag="dall", name=f"dall_{s}")
                rall8 = rpool.tile([P, 64], f32, tag="rall", name=f"rall_{s}")
                rdram = drpool.tile([P, 64], f32, tag="rdram", name=f"rdram_{s}")
                return zun, zt, dall8, rall8, rdram

            # ---- schedule ----
            from itertools import chain

            emit_w("wq", WQT)
            emit_w("wk", WKT)
            emit_b("bq", BQ)
            emit_b("bk", BK)
            xt0 = load_x(0)
            emit_w("wv", WVT)
            emit_w("wo", WOT)
            emit_b("bo", BO)
            xt1 = load_x(1)

            q0 = qkpool.tile([P, NDT * N], bf16, tag="qt", name="qt_0")
            k0 = qkpool.tile([P, NDT * N], bf16, tag="kt", name="kt_0")
            q1 = qkpool.tile([P, NDT * N], bf16, tag="qt", name="qt_1")
            k1 = qkpool.tile([P, NDT * N], bf16, tag="kt", name="kt_1")

            # startup (ACT idle): first QK chunk + BOTH slices' V projections
            proj_qk_chunk(0, 0, xt0, q0, k0)
            v0 = proj_v(0, xt0)
            v1 = proj_v(1, xt1)
            a0 = alloc_attn(0)
            a1 = alloc_attn(1)

            # filler chain: exactly 8 pairs x 16 kv-steps = 128 matmuls
            F = chain(
                gen_qk_chunk(0, 1, xt0, q0, k0),
                gen_qk_chunk(0, 2, xt0, q0, k0),
                gen_qk_chunk(0, 3, xt0, q0, k0),
                gen_qk_chunk(1, 0, xt1, q1, k1),
                gen_qk_chunk(1, 1, xt1, q1, k1),
                gen_qk_chunk(1, 2, xt1, q1, k1),
                gen_qk_chunk(1, 3, xt1, q1, k1),
                gen_op_chunk(0, 0, a0[1]),
                gen_op_chunk(0, 1, a0[1]),
            )

            for j in range(NDT):
                attention_pair(0, j, q0, k0, v0, a0[0], a0[2], filler=F)
                norm_pair(0, j, a0[0], a0[2], a0[3], a0[4], a0[1])
            for j in range(NDT - 1):
                attention_pair(1, j, q1, k1, v1, a1[0], a1[2], filler=F)
                norm_pair(1, j, a1[0], a1[2], a1[3], a1[4], a1[1])
            attention_pair(1, 3, q1, k1, v1, a1[0], a1[2], filler=F)

            # tail: PE work first, then the last norm chain, so the DVE's
            # in-order queue doesn't head-of-line-block the out-proj evacs
            for _ in F:  # drain any leftover fillers
                pass
            for _ in gen_op_chunk(0, 2, a0[1]):
                pass
            for _ in gen_op_chunk(0, 3, a0[1]):
                pass
            norm_pair(1, 3, a1[0], a1[2], a1[3], a1[4], a1[1])
            out_proj_tail(1, (0, 1), a1[1])
            out_proj_tail(1, (2, 3), a1[1])

    nc.compile()
    return nc


def _get_nc():
    if "nc" not in _CACHE:
        _CACHE["nc"] = _build_nc()
    return _CACHE["nc"]


def kernel(X, Wq, bq, Wk, bk, Wv, bv, Wo, bo):
    from concourse.bass_utils import run_bass_kernel_spmd

    nc = _get_nc()
    bf16 = ml_dtypes.bfloat16

    Xf = np.asarray(X, np.float32).reshape(B * T, N, D)
    XT_all = np.ascontiguousarray(Xf.transpose(0, 2, 1)).astype(bf16)  # [16, D, N]
    WQT = np.ascontiguousarray(np.asarray(Wq, np.float32).T * S_SCALE).astype(bf16)
    WKT = np.ascontiguousarray(np.asarray(Wk, np.float32).T).astype(bf16)
    WVT = np.ascontiguousarray(np.asarray(Wv, np.float32).T).astype(bf16)
    WOT = np.ascontiguousarray(np.asarray(Wo, np.float32).T).astype(bf16)
    bo_eff = (np.asarray(bo, np.float32)
              + np.asarray(Wo, np.float32) @ np.asarray(bv, np.float32))
    BQa = (np.asarray(bq, np.float32) * S_SCALE).reshape(NDT, P, 1)
    BKa = np.asarray(bk, np.float32).reshape(NDT, P, 1)
    BOa = bo_eff.reshape(NDT, P, 1)

    in_maps = []
    for c in range(NCORES):
        in_maps.append({
            "XT": np.ascontiguousarray(XT_all[c * NSLICE:(c + 1) * NSLICE]),
            "WQT": WQT, "WKT": WKT, "WVT": WVT, "WOT": WOT,
            "BQ": BQa, "BK": BKa, "BO": BOa,
        })

    trace = bool(int(os.environ.get("KERNEL_TRACE", "0")))
    kwargs = {}
    if trace:
        import tempfile
        kwargs = {"trace": True, "tmpdir": tempfile.mkdtemp(prefix="ker_trace_")}
    res = run_bass_kernel_spmd(nc, in_maps, core_ids=list(range(NCORES)), **kwargs)
    _CACHE["last_exec_ns"] = res.exec_time_ns

    out = np.empty((B * T, N, D), np.float32)
    for c in range(NCORES):
        ot = np.asarray(res.results[c]["OT"], np.float32)  # [NSLICE, D, N]
        out[c * NSLICE:(c + 1) * NSLICE] = ot.transpose(0, 2, 1)
    return out.reshape(B, T, N, D)


# revision 20
# speedup vs baseline: 2.4110x; 1.0048x over previous
"""Multi-head self-attention (AdaptiveTemporalContrastEnhancement) on 8 TRN2 cores.

Key facts baked in:
- The temporal-difference bias delta_c is added uniformly along the softmax
  axis, so softmax cancels it exactly -> it is skipped entirely.
- max |logit| ~ 1.9, so softmax runs without max-subtraction.
- V bias + output bias fold into one effective output bias:
      out = A@(XWv^T + bv) Wo^T + bo = A@(XWv^T)Wo^T + (Wo bv + bo).
- 1/sqrt(dh) is folded into WQT/BQ host-side.
- Data parallel over the 16 (b,t) slices: 2 slices per core, no collectives.
- All matmuls in bf16 (1 cyc/row on PE); accumulation fp32 in PSUM.

Device layout per slice (all "T" = dim-major, tokens along the free axis):
  XT  [d, n]   : 4 x [128, 1024] sbuf tiles (host pre-transposed)
  QT,KT [e, n] : computed as W^T.T @ XT  (4 x [128,1024])
  V_pad [n, .] : token-major, padded per head to a [128,128] stationary:
                 even head h: V cols 0-63, ones col 64, zeros 65-127
                 odd  head h: ones col 0, zeros 1-63,  V cols 64-127
                 so the PV matmul puts head h's Z^T at partitions 64*(h%2)..+63
                 and the softmax denominator at row 64 (even) / row 0 (odd).
  S^T [kv, q]  : head-PAIR packed: one [128, 1024] psum tile holds both heads'
                 S^T for one (kv, q-half); the two S matmuls use disjoint PE
                 row groups (partitions 0-63 / 64-127) and run concurrently.
  Z^T [d, q]   : per (head, q-half) [128, 512] psum accum over kv; evacuated
                 (with denominator row) to sbuf zun per head block.
  denominators : reshaped by DMA into dall8[128, 64] (head h = 16 partitions
                 x 64 cols) so ONE short-free-dim reciprocal per head PAIR is
                 cheap; broadcast back via a DRAM bounce; TT-mult per head.
  O^T [e, n]   : out-proj from normalized Z^T; host transposes back.

The schedule is software-pipelined at instruction level: projection and
out-projection chunks are emitted between attention head-pairs so the PE
fills the idle left by the ACT-paced exp stream, keeping the PE busy (and
its HAM clock-gate warm) while both slices' attention runs back-to-back.
"""

import os
import numpy as np
import ml_dtypes

B, T, N, D = 2, 8, 1024, 512
H, DH = 8, 64
P = 128
NDT = D // P          # 4 d-tiles
NKV = N // P          # 8 kv tiles
NQH = N // 512        # 2 q halves
NCORES = 8
NSLICE = (B * T) // NCORES   # 2 slices per core
S_SCALE = float(1.0 / np.sqrt(DH))  # 0.125

_CACHE = {}


def _build_nc():
    import concourse.mybir as mybir
    from concourse import bacc
    from concourse.tile import TileContext
    import concourse.bass as bass

    f32, bf16 = mybir.dt.float32, mybir.dt.bfloat16
    nc = bacc.Bacc("TRN2", target_bir_lowering=False, debug=False)

    XT = nc.dram_tensor("XT", [NSLICE, D, N], bf16, kind="ExternalInput")
    WQT = nc.dram_tensor("WQT", [D, D], bf16, kind="ExternalInput")
    WKT = nc.dram_tensor("WKT", [D, D], bf16, kind="ExternalInput")
    WVT = nc.dram_tensor("WVT", [D, D], bf16, kind="ExternalInput")
    WOT = nc.dram_tensor("WOT", [D, D], bf16, kind="ExternalInput")
    BQ = nc.dram_tensor("BQ", [NDT, P, 1], f32, kind="ExternalInput")
    BK = nc.dram_tensor("BK", [NDT, P, 1], f32, kind="ExternalInput")
    BO = nc.dram_tensor("BO", [NDT, P, 1], f32, kind="ExternalInput")
    OT = nc.dram_tensor("OT", [NSLICE, D, N], f32, kind="ExternalOutput")

    Exp = mybir.ActivationFunctionType.Exp
    Mult = mybir.AluOpType.mult

    with TileContext(nc) as tc:
        with (
            tc.tile_pool(name="wpool", bufs=1) as wpool,
            tc.tile_pool(name="xpool", bufs=2) as xpool,
            tc.tile_pool(name="qkpool", bufs=2) as qkpool,
            tc.tile_pool(name="vpool", bufs=2) as vpool,
            tc.tile_pool(name="apool", bufs=4) as apool,
            tc.tile_pool(name="zpool", bufs=2) as zpool,
            tc.tile_pool(name="rpool", bufs=2) as rpool,
            tc.tile_pool(name="opool", bufs=3) as opool,
            tc.tile_pool(name="drpool", bufs=2, space="DRAM") as drpool,
            tc.tile_pool(name="ps_s", bufs=2, space="PSUM") as ps_s,
            tc.tile_pool(name="ps_z", bufs=2, space="PSUM") as ps_z,
            tc.tile_pool(name="ps_c", bufs=1, space="PSUM") as ps_c,
        ):
            # ---- persistent weights / biases (DMAs split for queue parallelism;
            #      wq/wk/bq/bk first so the first projection chunk starts early) ----
            w_sb, b_sb = {}, {}

            def emit_w(name, dram):
                # one 3D-AP DMA per weight: [512,512] dram -> [128, 4*512] sbuf
                t = wpool.tile([P, NDT * 512], bf16, tag=name, name=f"w_{name}")
                w_sb[name] = t
                nc.sync.dma_start(
                    out=t[:, :].rearrange("p (dt e) -> p dt e", e=512),
                    in_=dram[:, :].rearrange("(dt p) e -> p dt e", p=P),
                )

            def emit_b(name, dram):
                t = wpool.tile([P, NDT], f32, tag=name, name=f"b_{name}")
                b_sb[name] = t
                nc.sync.dma_start(
                    out=t[:, :],
                    in_=dram[:, :, :].rearrange("et p one -> p (et one)"),
                )

            def load_x(s):
                # one 1MB DMA: large transfers reach full fabric bandwidth
                xt = xpool.tile([P, NDT * N], bf16, tag="xt", name=f"xt_{s}")
                nc.sync.dma_start(
                    out=xt[:, :].rearrange("p (dt n) -> p dt n", n=N),
                    in_=XT[s].rearrange("(dt p) n -> p dt n", p=P),
                )
                return xt

            def gen_qk_chunk(s, et, xt, qt, kt):
                """Filler generator: yields after each matmul so attention
                can weave these into the exp-paced stream one MM at a time."""
                for dst, wname, bname in ((qt[et], "wq", "bq"), (kt[et], "wk", "bk")):
                    w = w_sb[wname]
                    ps = ps_c.tile([P, N], f32, tag="c", name=f"psc_{wname}_{s}_{et}")
                    for dt_ in range(NDT):
                        for qh in range(NQH):
                            nc.tensor.matmul(
                                ps[:, qh * 512:(qh + 1) * 512],
                                lhsT=w[:, dt_ * 512 + et * P: dt_ * 512 + (et + 1) * P],
                                rhs=xt[:, dt_ * N + qh * 512: dt_ * N + qh * 512 + 512],
                                start=(dt_ == 0), stop=(dt_ == NDT - 1),
                            )
                            if dt_ == NDT - 1:
                                # evac inside the same pop as the last matmul so
                                # consumers emitted next step see it ordered
                                nc.vector.tensor_scalar_add(
                                    dst[:, qh * 512: qh * 512 + 512],
                                    ps[:, qh * 512:(qh + 1) * 512],
                                    b_sb[bname][:, et:et + 1],
                                )
                            yield

            def gen_op_chunk(s, et, zt):
                ps = ps_c.tile([P, N], f32, tag="c", name=f"psc_o_{s}_{et}")
                o_sb = opool.tile([P, N], f32, tag="o", name=f"o_{s}_{et}")
                for dd in range(NDT):
                    for qh in range(NQH):
                        nc.tensor.matmul(
                            ps[:, qh * 512:(qh + 1) * 512],
                            lhsT=w_sb["wo"][:, dd * 512 + et * P: dd * 512 + (et + 1) * P],
                            rhs=zt[dd][:, qh * 512: qh * 512 + 512],
                            start=(dd == 0), stop=(dd == NDT - 1),
                        )
                        if dd == NDT - 1:
                            nc.vector.tensor_scalar_add(
                                o_sb[:, qh * 512:(qh + 1) * 512],
                                ps[:, qh * 512:(qh + 1) * 512], b_sb["bo"][:, et:et + 1])
                            if qh == NQH - 1:
                                nc.sync.dma_start(
                                    out=OT[s, et * P:(et + 1) * P, :], in_=o_sb)
                        yield

            def proj_qk_chunk(s, et, xt, qt, kt):
                for dst, wname, bname in ((qt[et], "wq", "bq"), (kt[et], "wk", "bk")):
                    w = w_sb[wname]
                    ps = ps_s.tile([P, N], f32, tag="s", name=f"ps_{wname}_{s}_{et}")
                    for dt_ in range(NDT):
                        for qh in range(NQH):
                            nc.tensor.matmul(
                                ps[:, qh * 512:(qh + 1) * 512],
                                lhsT=w[:, dt_ * 512 + et * P: dt_ * 512 + (et + 1) * P],
                                rhs=xt[:, dt_ * N + qh * 512: dt_ * N + qh * 512 + 512],
                                start=(dt_ == 0), stop=(dt_ == NDT - 1),
                            )
                    for qh in range(NQH):
                        nc.vector.tensor_scalar_add(
                            dst[:, qh * 512: qh * 512 + 512],
                            ps[:, qh * 512:(qh + 1) * 512],
                            b_sb[bname][:, et:et + 1],
                        )

            def proj_v(s, xt):
                v_sb = vpool.tile([P, NKV * H * P], bf16, tag="v", name=f"v_{s}")
                vz = v_sb.rearrange("p (b r) -> p b r", r=256)
                nc.gpsimd.memset(vz[:, :, 65:128], 0.0)    # even-head pad
                nc.gpsimd.memset(vz[:, :, 129:192], 0.0)   # odd-head pad
                nc.vector.memset(vz[:, :, 64:65], 1.0)     # even-head ones col
                nc.vector.memset(vz[:, :, 128:129], 1.0)   # odd-head ones col
                for kv in range(NKV):
                    ps = ps_s.tile([P, N], f32, tag="s", name=f"ps_v_{s}_{kv}")
                    for dt_ in range(NDT):
                        nc.tensor.matmul(
                            ps[:, 0:512],
                            lhsT=xt[:, dt_ * N + kv * P: dt_ * N + (kv + 1) * P],
                            rhs=w_sb["wv"][:, dt_ * 512:(dt_ + 1) * 512],
                            start=(dt_ == 0), stop=(dt_ == NDT - 1),
                        )
                    vblk = v_sb[:, kv * 1024:(kv + 1) * 1024].rearrange(
                        "p (hp r) -> p hp r", r=256)
                    psh = ps[:, 0:512].rearrange("p (hp c) -> p hp c", c=128)
                    nc.vector.tensor_copy(vblk[:, :, 0:64], psh[:, :, 0:64])
                    nc.vector.tensor_copy(vblk[:, :, 192:256], psh[:, :, 64:128])
                return v_sb

            def attention_pair(s, j, qt, kt, v_sb, zun, dall8, filler=None):
                """Heads 2j, 2j+1: S matmuls packed into disjoint PE row
                groups; one exp covers both heads; PV per head/q-half. One
                filler matmul is woven in after each kv step."""
                et = j
                for qh in range(NQH):
                    zs = [ps_z.tile([P, 512], f32, tag="z", name=f"z_{s}_{j}_{qh}_{p_}")
                          for p_ in range(2)]
                    for kv in range(NKV):
                        s_ps = ps_s.tile([P, N], f32, tag="s", name=f"s_{s}_{j}_{qh}_{kv}")
                        for p_ in range(2):
                            pb = 64 * p_
                            nc.tensor.matmul(
                                s_ps[:, p_ * 512:(p_ + 1) * 512],
                                lhsT=kt[et][pb:pb + 64, kv * P:(kv + 1) * P],
                                rhs=qt[et][pb:pb + 64, qh * 512: qh * 512 + 512],
                                start=True, stop=True,
                            )
                        at = apool.tile([P, N], bf16, tag="at", name=f"at_{s}_{j}_{qh}_{kv}")
                        nc.scalar.activation(at, s_ps, Exp)
                        for p_ in range(2):
                            h = 2 * j + p_
                            nc.tensor.matmul(
                                zs[p_],
                                lhsT=v_sb[:, kv * 1024 + h * P: kv * 1024 + (h + 1) * P],
                                rhs=at[:, p_ * 512:(p_ + 1) * 512],
                                start=(kv == 0), stop=(kv == NKV - 1),
                            )
                        if filler is not None:
                            next(filler, None)
                    for p_ in range(2):
                        h = 2 * j + p_
                        nc.vector.tensor_copy(
                            zun[:, h * N + qh * 512: h * N + qh * 512 + 512], zs[p_])
                for p_ in range(2):
                    h = 2 * j + p_
                    dr = 64 if h % 2 == 0 else 0
                    # reshape-gather the denom row into dall8[16h:16h+16, 0:64]
                    nc.gpsimd.dma_start(  # gpsimd: casting DMA bf16 -> f32
                        out=dall8[16 * h:16 * (h + 1), :],
                        in_=zun[dr:dr + 1, h * N:(h + 1) * N])

            def norm_pair(s, j, zun, dall8, rall8, rdram, zt):
                # short-free-dim reciprocal over the pair's 32 partitions
                nc.vector.reciprocal(rall8[32 * j:32 * (j + 1), :],
                                     dall8[32 * j:32 * (j + 1), :])
                nc.sync.dma_start(out=rdram[32 * j:32 * (j + 1), :],
                                  in_=rall8[32 * j:32 * (j + 1), :])
                rbc = rpool.tile([P, N], f32, tag="rbc", name=f"rbc_{s}_{j}")
                for p_ in range(2):
                    h = 2 * j + p_
                    base = rdram[0:1, 0:1]
                    nc.sync.dma_start(
                        out=rbc[64 * p_:64 * p_ + 64, :],
                        in_=bass.AP(tensor=base.tensor, offset=base.offset + h * N,
                                    ap=[[0, 64], [1, N]]),
                    )
                for p_ in range(2):
                    h = 2 * j + p_
                    pb = 64 * p_
                    nc.vector.tensor_tensor(
                        out=zt[j][pb:pb + 64, :],
                        in0=zun[pb:pb + 64, h * N:(h + 1) * N],
                        in1=rbc[pb:pb + 64, :], op=Mult,
                    )

            def out_proj_tail(s, ets, zt):
                """Out-proj for two e-tiles with the dd=3 (last head pair)
                contraction deferred, so these matmuls start before the last
                norm_pair's TT-mults have produced zt block 3."""
                pss = {}
                for et in ets:
                    ps = ps_s.tile([P, N], f32, tag="s", name=f"ps_ot_{s}_{et}")
                    pss[et] = ps
                    for dd in range(NDT - 1):
                        for qh in range(NQH):
                            nc.tensor.matmul(
                                ps[:, qh * 512:(qh + 1) * 512],
                                lhsT=w_sb["wo"][:, dd * 512 + et * P: dd * 512 + (et + 1) * P],
                                rhs=zt[dd][:, qh * 512: qh * 512 + 512],
                                start=(dd == 0), stop=False,
                            )
                for et in ets:
                    ps = pss[et]
                    dd = NDT - 1
                    for qh in range(NQH):
                        nc.tensor.matmul(
                            ps[:, qh * 512:(qh + 1) * 512],
                            lhsT=w_sb["wo"][:, dd * 512 + et * P: dd * 512 + (et + 1) * P],
                            rhs=zt[dd][:, qh * 512: qh * 512 + 512],
                            start=False, stop=True,
                        )
                    o_sb = opool.tile([P, N], f32, tag="o", name=f"o_{s}_{et}")
                    for qh in range(NQH):
                        nc.vector.tensor_scalar_add(
                            o_sb[:, qh * 512:(qh + 1) * 512],
                            ps[:, qh * 512:(qh + 1) * 512], b_sb["bo"][:, et:et + 1])
                    nc.sync.dma_start(out=OT[s, et * P:(et + 1) * P, :], in_=o_sb)

            def out_proj_chunk(s, et, zt):
                ps = ps_s.tile([P, N], f32, tag="s", name=f"ps_o_{s}_{et}")
                for dd in range(NDT):
                    for qh in range(NQH):
                        nc.tensor.matmul(
                            ps[:, qh * 512:(qh + 1) * 512],
                            lhsT=w_sb["wo"][:, dd * 512 + et * P: dd * 512 + (et + 1) * P],
                            rhs=zt[:, dd * N + qh * 512: dd * N + qh * 512 + 512],
                            start=(dd == 0), stop=(dd == NDT - 1),
                        )
                o_sb = opool.tile([P, N], f32, tag="o", name=f"o_{s}_{et}")
                for qh in range(NQH):
                    nc.vector.tensor_scalar_add(
                        o_sb[:, qh * 512:(qh + 1) * 512],
                        ps[:, qh * 512:(qh + 1) * 512], b_sb["bo"][:, et:et + 1])
                nc.sync.dma_start(out=OT[s, et * P:(et + 1) * P, :], in_=o_sb)

            def alloc_attn(s):
                zun = zpool.tile([P, H * N], bf16, tag="zun", name=f"zun_{s}")
                zt = [zpool.tile([P, N], bf16, tag=f"zt{j}", name=f"zt_{s}_{j}")
                      for j in range(NDT)]
                dall8 = rpool.tile([P, 64], f32, tag="dall", name=f"dall_{s}")
                rall8 = rpool.tile([P, 64], f32, tag="rall", name=f"rall_{s}")
                rdram = drpool.tile([P, 64], f32, tag="rdram", name=f"rdram_{s}")
                return zun, zt, dall8, rall8, rdram

            # ---- schedule ----
            from itertools import chain

            emit_w("wq", WQT)
            xt0 = load_x(0)
            emit_w("wk", WKT)
            emit_b("bq", BQ)
            emit_b("bk", BK)
            emit_w("wv", WVT)
            emit_w("wo", WOT)
            emit_b("bo", BO)
            xt1 = load_x(1)

            q0 = [qkpool.tile([P, N], bf16, tag=f"qt{j}", name=f"qt_0_{j}") for j in range(NDT)]
            k0 = [qkpool.tile([P, N], bf16, tag=f"kt{j}", name=f"kt_0_{j}") for j in range(NDT)]
            q1 = [qkpool.tile([P, N], bf16, tag=f"qt{j}", name=f"qt_1_{j}") for j in range(NDT)]
            k1 = [qkpool.tile([P, N], bf16, tag=f"kt{j}", name=f"kt_1_{j}") for j in range(NDT)]

            # startup (ACT idle): first QK chunk + BOTH slices' V projections
            proj_qk_chunk(0, 0, xt0, q0, k0)
            v0 = proj_v(0, xt0)
            v1 = proj_v(1, xt1)
            a0 = alloc_attn(0)
            a1 = alloc_attn(1)

            # filler chain: exactly 8 pairs x 16 kv-steps = 128 matmuls
            F = chain(
                gen_qk_chunk(0, 1, xt0, q0, k0),
                gen_qk_chunk(0, 2, xt0, q0, k0),
                gen_qk_chunk(0, 3, xt0, q0, k0),
                gen_qk_chunk(1, 0, xt1, q1, k1),
                gen_qk_chunk(1, 1, xt1, q1, k1),
                gen_qk_chunk(1, 2, xt1, q1, k1),
                gen_qk_chunk(1, 3, xt1, q1, k1),
                gen_op_chunk(0, 0, a0[1]),
                gen_op_chunk(0, 1, a0[1]),
            )

            for j in range(NDT):
                attention_pair(0, j, q0, k0, v0, a0[0], a0[2], filler=F)
                norm_pair(0, j, a0[0], a0[2], a0[3], a0[4], a0[1])
            for j in range(NDT - 1):
                attention_pair(1, j, q1, k1, v1, a1[0], a1[2], filler=F)
                norm_pair(1, j, a1[0], a1[2], a1[3], a1[4], a1[1])
            attention_pair(1, 3, q1, k1, v1, a1[0], a1[2], filler=F)

            # tail: PE work first, then the last norm chain, so the DVE's
            # in-order queue doesn't head-of-line-block the out-proj evacs
            for _ in F:  # drain any leftover fillers
                pass
            for _ in gen_op_chunk(0, 2, a0[1]):
                pass
            for _ in gen_op_chunk(0, 3, a0[1]):
                pass
            norm_pair(1, 3, a1[0], a1[2], a1[3], a1[4], a1[1])
            out_proj_tail(1, (0, 1), a1[1])
            out_proj_tail(1, (2, 3), a1[1])

    nc.compile()
    return nc


def _get_nc():
    if "nc" not in _CACHE:
        _CACHE["nc"] = _build_nc()
    return _CACHE["nc"]


def kernel(X, Wq, bq, Wk, bk, Wv, bv, Wo, bo):
    from concourse.bass_utils import run_bass_kernel_spmd

    nc = _get_nc()
    bf16 = ml_dtypes.bfloat16

    Xf = np.asarray(X, np.float32).reshape(B * T, N, D)
    XT_all = np.ascontiguousarray(Xf.transpose(0, 2, 1)).astype(bf16)  # [16, D, N]
    WQT = np.ascontiguousarray(np.asarray(Wq, np.float32).T * S_SCALE).astype(bf16)
    WKT = np.ascontiguousarray(np.asarray(Wk, np.float32).T).astype(bf16)
    WVT = np.ascontiguousarray(np.asarray(Wv, np.float32).T).astype(bf16)
    WOT = np.ascontiguousarray(np.asarray(Wo, np.float32).T).astype(bf16)
    bo_eff = (np.asarray(bo, np.float32)
              + np.asarray(Wo, np.float32) @ np.asarray(bv, np.float32))
    BQa = (np.asarray(bq, np.float32) * S_SCALE).reshape(NDT, P, 1)
    BKa = np.asarray(bk, np.float32).reshape(NDT, P, 1)
    BOa = bo_eff.reshape(NDT, P, 1)

    in_maps = []
    for c in range(NCORES):
        in_maps.append({
            "XT": np.ascontiguousarray(XT_all[c * NSLICE:(c + 1) * NSLICE]),
            "WQT": WQT, "WKT": WKT, "WVT": WVT, "WOT": WOT,
            "BQ": BQa, "BK": BKa, "BO": BOa,
        })

    trace = bool(int(os.environ.get("KERNEL_TRACE", "0")))
    kwargs = {}
    if trace:
        import tempfile
        kwargs = {"trace": True, "tmpdir": tempfile.mkdtemp(prefix="ker_trace_")}
    res = run_bass_kernel_spmd(nc, in_maps, core_ids=list(range(NCORES)), **kwargs)
    _CACHE["last_exec_ns"] = res.exec_time_ns

    out = np.empty((B * T, N, D), np.float32)
    for c in range(NCORES):
        ot = np.asarray(res.results[c]["OT"], np.float32)  # [NSLICE, D, N]
        out[c * NSLICE:(c + 1) * NSLICE] = ot.transpose(0, 2, 1)
    return out.reshape(B, T, N, D)
